# revision 2
# baseline (speedup 1.0000x reference)
"""Causal self-attention (B=4, T=2048, C=1024, 16 heads) on 8 trn2 NeuronCores.

Sharding: core c = (batch c//2, head-group c%2 of 8 heads). Data-parallel over
batch, tensor-parallel over heads; out-proj is row-sharded and the two partial
products per batch are summed on the host (no device collectives).

Device program per core (all fp32):
  phase 1: q^T/k^T = W^T @ x^T as head-pair tiles [128, T] (d on partitions)
  phase 2: V in natural [t, d] layout, augmented with a ones column per head
           (bias trick) so P@V also accumulates softmax row-sums for free
  phase 3: flash-style attention in S^T layout (S computed transposed — no PE
           transposes, no max subtraction: |S| < ~3 by construction), causal
           masking via a single static 128x128 triangular mask + memsets,
           normalization folded into the PSUM->SBUF copy
  phase 4: out = y^T-slices^T @ W_proj rows (partial over this core's heads)
"""

import os
import sys

import numpy as np

for _p in ("/opt/trn_rl_repo", "/root/.axon_site/_ro/trn_rl_repo"):
    if os.path.isdir(_p) and _p not in sys.path:
        sys.path.insert(0, _p)

import concourse.bass as bass  # noqa: E402
import concourse.tile as tile  # noqa: E402
from concourse import bacc, mybir  # noqa: E402
from concourse.bass_utils import run_bass_kernel_spmd  # noqa: E402

B, T, C = 4, 2048, 1024
H, D = 16, 64
N_CORES = 8
F32 = mybir.dt.float32
TC = T // 512  # 4 t-chunks of 512
TT = T // 128  # 16 t-tiles of 128
CT = C // 128  # 8 c-tiles of 128

_cache: dict = {}


def _emit(nc: "bacc.Bacc", tc: "tile.TileContext", d: dict) -> None:
    mult = mybir.AluOpType.mult
    add = mybir.AluOpType.add
    Exp = mybir.ActivationFunctionType.Exp
    dma = nc.sync.dma_start

    with (
        tc.tile_pool(name="const", bufs=1) as cpool,
        tc.tile_pool(name="persist", bufs=1) as persist,
    ):
        mask_sb = cpool.tile([128, 128], F32, name="mask", tag="mask")
        dma(out=mask_sb[:], in_=d["mask"].ap())
        bq_sb = cpool.tile([128, 4], F32, name="bq", tag="bq")
        dma(out=bq_sb[:], in_=d["bq"].ap())
        bk_sb = cpool.tile([128, 4], F32, name="bk", tag="bk")
        dma(out=bk_sb[:], in_=d["bk"].ap())
        bv_sb = cpool.tile([128, 520], F32, name="bv", tag="bv")
        dma(out=bv_sb[:], in_=d["bv"].ap())

        qT = [persist.tile([128, T], F32, name=f"qT{p}", tag=f"qT{p}") for p in range(4)]
        kT = [persist.tile([128, T], F32, name=f"kT{p}", tag=f"kT{p}") for p in range(4)]
        Vt = [persist.tile([128, 520], F32, name=f"V{i}", tag=f"V{i}") for i in range(TT)]
        yT = [persist.tile([128, T], F32, name=f"yT{p}", tag=f"yT{p}") for p in range(4)]

        # ---- phase 1: q^T / k^T head-pair tiles ----
        with (
            tc.tile_pool(name="wqk", bufs=1) as wpool,
            tc.tile_pool(name="xt1", bufs=2) as xpool,
            tc.tile_pool(name="psqk", bufs=4, space="PSUM") as pq,
        ):
            wq_sb = wpool.tile([128, CT, 512], F32, name="wq", tag="wq")
            dma(out=wq_sb[:], in_=d["wq"].ap().rearrange("(ct p) n -> p ct n", p=128))
            wk_sb = wpool.tile([128, CT, 512], F32, name="wk", tag="wk")
            dma(out=wk_sb[:], in_=d["wk"].ap().rearrange("(ct p) n -> p ct n", p=128))
            for tci in range(TC):
                xt = xpool.tile([128, CT, 512], F32, name="xt", tag="xt")
                dma(
                    out=xt[:],
                    in_=d["xT"].ap()[:, 512 * tci : 512 * tci + 512].rearrange(
                        "(ct p) n -> p ct n", p=128
                    ),
                )
                for p in range(4):
                    for w_sb, dstT, scale, bias_sb in (
                        (wq_sb, qT, 0.125, bq_sb),
                        (wk_sb, kT, 1.0, bk_sb),
                    ):
                        ps = pq.tile([128, 512], F32, name="psqk", tag="psqk")
                        for ct in range(CT):
                            nc.tensor.matmul(
                                ps[:],
                                w_sb[:, ct, 128 * p : 128 * p + 128],
                                xt[:, ct, :],
                                start=(ct == 0),
                                stop=(ct == CT - 1),
                            )
                        nc.vector.tensor_scalar(
                            dstT[p][:, 512 * tci : 512 * tci + 512],
                            ps[:],
                            scale,
                            bias_sb[:, p : p + 1],
                            mult,
                            add,
                        )

        # ---- phase 2: V (natural layout, ones-augmented) ----
        with (
            tc.tile_pool(name="wv", bufs=1) as wvpool,
            tc.tile_pool(name="xt2", bufs=2) as xpool2,
            tc.tile_pool(name="psv", bufs=4, space="PSUM") as pv,
        ):
            wv_sb = wvpool.tile([128, CT, 520], F32, name="wv", tag="wv")
            dma(out=wv_sb[:], in_=d["wv"].ap().rearrange("(ct p) n -> p ct n", p=128))
            for tci in range(TC):
                xt = xpool2.tile([128, CT, 512], F32, name="xt2", tag="xt2")
                dma(
                    out=xt[:],
                    in_=d["xT"].ap()[:, 512 * tci : 512 * tci + 512].rearrange(
                        "(ct p) n -> p ct n", p=128
                    ),
                )
                for tt in range(4):
                    for qd in range(2):
                        ps = pv.tile([128, 260], F32, name="psv", tag="psv")
                        for ct in range(CT):
                            nc.tensor.matmul(
                                ps[:],
                                xt[:, ct, 128 * tt : 128 * tt + 128],
                                wv_sb[:, ct, 260 * qd : 260 * qd + 260],
                                start=(ct == 0),
                                stop=(ct == CT - 1),
                            )
                        nc.vector.tensor_tensor(
                            Vt[4 * tci + tt][:, 260 * qd : 260 * qd + 260],
                            ps[:],
                            bv_sb[:, 260 * qd : 260 * qd + 260],
                            add,
                        )

        # ---- phase 3: attention, S^T layout ----
        with (
            tc.tile_pool(name="es", bufs=3) as espool,
            tc.tile_pool(name="rc", bufs=4) as rcpool,
            tc.tile_pool(name="pss", bufs=2, space="PSUM") as pss,
            tc.tile_pool(name="psy", bufs=2, space="PSUM") as psy,
        ):
            for hl in range(8):
                p, half = hl // 2, hl % 2
                pr = 64 * half
                for qc in range(4):
                    yq = psy.tile([65, 512], F32, name="yq", tag="yq")
                    nki = 4 * qc + 4
                    for blk in range(nki // 2):
                        sblk = pss.tile([128, 1024], F32, name="sblk", tag="sblk")
                        for j in (0, 1):
                            ki = 2 * blk + j
                            vs = max(0, 128 * (ki - 4 * qc))
                            sc = 512 * j
                            nc.tensor.matmul(
                                sblk[:, sc + vs : sc + 512],
                                kT[p][pr : pr + 64, 128 * ki : 128 * ki + 128],
                                qT[p][pr : pr + 64, 512 * qc + vs : 512 * qc + 512],
                                start=True,
                                stop=True,
                            )
                        es = espool.tile([128, 1024], F32, name="es", tag="es")
                        nc.scalar.activation(es[:], sblk[:], Exp)
                        for j in (0, 1):
                            ki = 2 * blk + j
                            r = ki - 4 * qc
                            sc = 512 * j
                            if r >= 0:
                                vs = 128 * r
                                if vs:
                                    nc.gpsimd.memset(es[:, sc : sc + vs], 0.0)
                                nc.vector.tensor_tensor(
                                    es[:, sc + vs : sc + vs + 128],
                                    es[:, sc + vs : sc + vs + 128],
                                    mask_sb[:],
                                    mult,
                                )
                        for j in (0, 1):
                            ki = 2 * blk + j
                            nc.tensor.matmul(
                                yq[:],
                                Vt[ki][:, 65 * hl : 65 * hl + 65],
                                es[:, 512 * j : 512 * j + 512],
                                start=(ki == 0),
                                stop=(ki == nki - 1),
                                skip_group_check=True,
                            )
                    rc = rcpool.tile([1, 512], F32, name="rc", tag="rc")
                    nc.vector.reciprocal(rc[:], yq[64:65, :])
                    rcb = rcpool.tile([64, 512], F32, name="rcb", tag="rcb")
                    nc.gpsimd.partition_broadcast(rcb[:], rc[:])
                    nc.vector.tensor_tensor(
                        yT[p][pr : pr + 64, 512 * qc : 512 * qc + 512],
                        yq[0:64, :],
                        rcb[:],
                        mult,
                    )

        # ---- phase 4: out-projection (row-sharded partial) ----
        with (
            tc.tile_pool(name="wpj", bufs=1) as wppool,
            tc.tile_pool(name="osb", bufs=3) as opool,
            tc.tile_pool(name="pso", bufs=4, space="PSUM") as pso,
        ):
            wp_sb = wppool.tile([128, 4, 1024], F32, name="wp", tag="wp")
            dma(out=wp_sb[:], in_=d["wproj"].ap().rearrange("(pp p) n -> p pp n", p=128))
            for tt in range(TT):
                for cc in range(2):
                    ps = pso.tile([128, 512], F32, name="pso", tag="pso")
                    for pp in range(4):
                        nc.tensor.matmul(
                            ps[:],
                            yT[pp][:, 128 * tt : 128 * tt + 128],
                            wp_sb[:, pp, 512 * cc : 512 * cc + 512],
                            start=(pp == 0),
                            stop=(pp == 3),
                        )
                    ob = opool.tile([128, 512], F32, name="ob", tag="ob")
                    nc.scalar.copy(ob[:], ps[:])
                    dma(
                        out=d["out"].ap()[
                            128 * tt : 128 * tt + 128, 512 * cc : 512 * cc + 512
                        ],
                        in_=ob[:],
                    )


def _build():
    nc = bacc.Bacc("TRN2", target_bir_lowering=False, debug=False, num_devices=N_CORES)
    d = {
        "xT": nc.dram_tensor("xT", [C, T], F32, kind="ExternalInput"),
        "wq": nc.dram_tensor("wq", [C, 512], F32, kind="ExternalInput"),
        "wk": nc.dram_tensor("wk", [C, 512], F32, kind="ExternalInput"),
        "wv": nc.dram_tensor("wv", [C, 520], F32, kind="ExternalInput"),
        "bv": nc.dram_tensor("bv", [128, 520], F32, kind="ExternalInput"),
        "bq": nc.dram_tensor("bq", [128, 4], F32, kind="ExternalInput"),
        "bk": nc.dram_tensor("bk", [128, 4], F32, kind="ExternalInput"),
        "mask": nc.dram_tensor("mask", [128, 128], F32, kind="ExternalInput"),
        "wproj": nc.dram_tensor("wproj", [512, 1024], F32, kind="ExternalInput"),
        "out": nc.dram_tensor("out", [T, C], F32, kind="ExternalOutput"),
    }
    with tile.TileContext(nc) as tcx:
        _emit(nc, tcx, d)
    nc.compile()
    return nc


def _prep_core_inputs(c, x, w_attn, b_attn):
    g = c % 2
    xT = np.ascontiguousarray(x[c // 2].T)
    wq = np.ascontiguousarray(w_attn[:, 512 * g : 512 * g + 512])
    wk = np.ascontiguousarray(w_attn[:, 1024 + 512 * g : 1024 + 512 * g + 512])
    wv = np.zeros((C, 520), np.float32)
    bv = np.zeros((128, 520), np.float32)
    for hl in range(8):
        hcol = 2048 + 512 * g + 64 * hl
        wv[:, 65 * hl : 65 * hl + 64] = w_attn[:, hcol : hcol + 64]
        bv[:, 65 * hl : 65 * hl + 64] = b_attn[hcol : hcol + 64][None, :]
        bv[:, 65 * hl + 64] = 1.0
    bq = np.zeros((128, 4), np.float32)
    bk = np.zeros((128, 4), np.float32)
    for p in range(4):
        bq[:, p] = b_attn[512 * g + 128 * p : 512 * g + 128 * p + 128] * 0.125
        bk[:, p] = b_attn[1024 + 512 * g + 128 * p : 1024 + 512 * g + 128 * p + 128]
    mask = (np.arange(128)[:, None] <= np.arange(128)[None, :]).astype(np.float32)
    return dict(xT=xT, wq=wq, wk=wk, wv=wv, bv=bv, bq=bq, bk=bk, mask=mask)


def make_in_maps(x, w_attn, b_attn, w_proj):
    x = np.asarray(x, np.float32)
    w_attn = np.asarray(w_attn, np.float32)
    b_attn = np.asarray(b_attn, np.float32)
    w_proj = np.asarray(w_proj, np.float32)
    in_maps = []
    for c in range(N_CORES):
        m = _prep_core_inputs(c, x, w_attn, b_attn)
        g = c % 2
        m["wproj"] = np.ascontiguousarray(w_proj[512 * g : 512 * g + 512, :])
        in_maps.append(m)
    return in_maps


def get_nc():
    if "nc" not in _cache:
        _cache["nc"] = _build()
    return _cache["nc"]


def gather(results, b_proj):
    b_proj = np.asarray(b_proj, np.float32)
    full = np.empty((B, T, C), np.float32)
    for b in range(B):
        full[b] = results[2 * b]["out"] + results[2 * b + 1]["out"] + b_proj[None, :]
    return full


def kernel(x, w_attn, b_attn, w_proj, b_proj):
    nc = get_nc()
    in_maps = make_in_maps(x, w_attn, b_attn, w_proj)
    res = run_bass_kernel_spmd(nc, in_maps, list(range(N_CORES)))
    return gather(res.results, b_proj)


# revision 6
# speedup vs baseline: 2.0322x; 2.0322x over previous
"""Causal self-attention (B=4, T=2048, C=1024, 16 heads) on 8 trn2 NeuronCores.

Sharding: core c = (batch c//2, head-group c%2 of 8 heads). Data-parallel over
batch, tensor-parallel over heads; out-proj is row-sharded and the two partial
products per batch are summed on the host (no device collectives).

Device program per core (all fp32):
  phase 1: q^T/k^T = W^T @ x^T as head-pair tiles [128, T] (d on partitions)
  phase 2: V in natural [t, d] layout, augmented with a ones column per head
           (bias trick) so P@V also accumulates softmax row-sums for free
  phase 3: flash-style attention in S^T layout (S computed transposed — no PE
           transposes, no max subtraction: |S| < ~3 by construction), causal
           masking via a single static 128x128 triangular mask + memsets,
           normalization folded into the PSUM->SBUF copy
  phase 4: out = y^T-slices^T @ W_proj rows (partial over this core's heads)
"""

import os
import sys

import numpy as np

for _p in ("/opt/trn_rl_repo", "/root/.axon_site/_ro/trn_rl_repo"):
    if os.path.isdir(_p) and _p not in sys.path:
        sys.path.insert(0, _p)

import concourse.bass as bass  # noqa: E402
import concourse.tile as tile  # noqa: E402
from concourse import bacc, mybir  # noqa: E402
from concourse.bass_utils import run_bass_kernel_spmd  # noqa: E402

B, T, C = 4, 2048, 1024
H, D = 16, 64
N_CORES = 8
F32 = mybir.dt.float32
F32R = mybir.dt.float32r


def _r(ap):
    return ap.bitcast(F32R)
TC = T // 512  # 4 t-chunks of 512
TT = T // 128  # 16 t-tiles of 128
CT = C // 128  # 8 c-tiles of 128

_cache: dict = {}


def _emit(nc: "bacc.Bacc", tc: "tile.TileContext", d: dict) -> None:
    mult = mybir.AluOpType.mult
    add = mybir.AluOpType.add
    Exp = mybir.ActivationFunctionType.Exp
    dma = nc.sync.dma_start

    with (
        tc.tile_pool(name="const", bufs=1) as cpool,
        tc.tile_pool(name="persist", bufs=1) as persist,
    ):
        mask_sb = cpool.tile([128, 128], F32R, name="mask", tag="mask")
        dma(out=mask_sb[:], in_=d["mask"].ap())
        bq_sb = cpool.tile([128, 4], F32, name="bq", tag="bq")
        dma(out=bq_sb[:], in_=d["bq"].ap())
        bk_sb = cpool.tile([128, 4], F32, name="bk", tag="bk")
        dma(out=bk_sb[:], in_=d["bk"].ap())
        bv_sb = cpool.tile([128, 520], F32, name="bv", tag="bv")
        dma(out=bv_sb[:], in_=d["bv"].ap())

        qT = [persist.tile([128, T], F32R, name=f"qT{p}", tag=f"qT{p}") for p in range(4)]
        kT = [persist.tile([128, T], F32R, name=f"kT{p}", tag=f"kT{p}") for p in range(4)]
        Vt = [persist.tile([128, 520], F32R, name=f"V{i}", tag=f"V{i}") for i in range(TT)]
        yT = [persist.tile([128, T], F32R, name=f"yT{p}", tag=f"yT{p}") for p in range(4)]

        # ---- phase 1: q^T / k^T head-pair tiles ----
        with (
            tc.tile_pool(name="wqk", bufs=1) as wpool,
            tc.tile_pool(name="xt1", bufs=2) as xpool,
            tc.tile_pool(name="psqk", bufs=4, space="PSUM") as pq,
        ):
            wq_sb = wpool.tile([128, CT, 512], F32R, name="wq", tag="wq")
            dma(out=wq_sb[:], in_=d["wq"].ap().rearrange("(ct p) n -> p ct n", p=128))
            wk_sb = wpool.tile([128, CT, 512], F32R, name="wk", tag="wk")
            dma(out=wk_sb[:], in_=d["wk"].ap().rearrange("(ct p) n -> p ct n", p=128))
            for tci in range(TC):
                xt = xpool.tile([128, CT, 512], F32R, name="xt", tag="xt")
                dma(
                    out=xt[:],
                    in_=d["xT"].ap()[:, 512 * tci : 512 * tci + 512].rearrange(
                        "(ct p) n -> p ct n", p=128
                    ),
                )
                for p in range(4):
                    for w_sb, dstT, scale, bias_sb in (
                        (wq_sb, qT, 0.125, bq_sb),
                        (wk_sb, kT, 1.0, bk_sb),
                    ):
                        ps = pq.tile([128, 512], F32, name="psqk", tag="psqk")
                        for ct in range(CT):
                            nc.tensor.matmul(
                                ps[:],
                                w_sb[:, ct, 128 * p : 128 * p + 128],
                                xt[:, ct, :],
                                start=(ct == 0),
                                stop=(ct == CT - 1),
                            )
                        nc.vector.tensor_scalar(
                            dstT[p][:, 512 * tci : 512 * tci + 512],
                            ps[:],
                            scale,
                            bias_sb[:, p : p + 1],
                            mult,
                            add,
                        )

        # ---- phase 2: V (natural layout, ones-augmented) ----
        with (
            tc.tile_pool(name="wv", bufs=1) as wvpool,
            tc.tile_pool(name="xt2", bufs=2) as xpool2,
            tc.tile_pool(name="psv", bufs=4, space="PSUM") as pv,
        ):
            wv_sb = wvpool.tile([128, CT, 520], F32R, name="wv", tag="wv")
            dma(out=wv_sb[:], in_=d["wv"].ap().rearrange("(ct p) n -> p ct n", p=128))
            for tci in range(TC):
                xt = xpool2.tile([128, CT, 512], F32R, name="xt2", tag="xt2")
                dma(
                    out=xt[:],
                    in_=d["xT"].ap()[:, 512 * tci : 512 * tci + 512].rearrange(
                        "(ct p) n -> p ct n", p=128
                    ),
                )
                for tt in range(4):
                    for qd in range(2):
                        ps = pv.tile([128, 260], F32, name="psv", tag="psv")
                        for ct in range(CT):
                            nc.tensor.matmul(
                                ps[:],
                                xt[:, ct, 128 * tt : 128 * tt + 128],
                                wv_sb[:, ct, 260 * qd : 260 * qd + 260],
                                start=(ct == 0),
                                stop=(ct == CT - 1),
                            )
                        nc.vector.tensor_tensor(
                            Vt[4 * tci + tt][:, 260 * qd : 260 * qd + 260],
                            ps[:],
                            bv_sb[:, 260 * qd : 260 * qd + 260],
                            add,
                        )

        # ---- phase 3: attention, S^T layout ----
        with (
            tc.tile_pool(name="es", bufs=3) as espool,
            tc.tile_pool(name="rc", bufs=8) as rcpool,
            tc.tile_pool(name="pss", bufs=2, space="PSUM") as pss,
            tc.tile_pool(name="psy", bufs=3, space="PSUM") as psy,
        ):
            for hl in range(8):
                p, half = hl // 2, hl % 2
                pr = 64 * half
                for qc in range(4):
                    yq = psy.tile([65, 512], F32, name="yq", tag="yq")
                    nki = 4 * qc + 4
                    for blk in range(nki // 2):
                        sblk = pss.tile([128, 1024], F32, name="sblk", tag="sblk")
                        for j in (0, 1):
                            ki = 2 * blk + j
                            vs = min(max(0, 128 * (ki - 4 * qc)), 256)
                            sc = 512 * j
                            nc.tensor.matmul(
                                sblk[:, sc + vs : sc + 512],
                                kT[p][pr : pr + 64, 128 * ki : 128 * ki + 128],
                                qT[p][pr : pr + 64, 512 * qc + vs : 512 * qc + 512],
                                start=True,
                                stop=True,
                            )
                        es = espool.tile([128, 1024], F32R, name="es", tag="es")
                        nc.scalar.activation(es[:], sblk[:], Exp)
                        for j in (0, 1):
                            ki = 2 * blk + j
                            r = ki - 4 * qc
                            sc = 512 * j
                            if r >= 0:
                                vs = 128 * r
                                if vs:
                                    nc.gpsimd.memset(es[:, sc : sc + vs].bitcast(F32), 0.0)
                                nc.vector.tensor_tensor(
                                    es[:, sc + vs : sc + vs + 128],
                                    es[:, sc + vs : sc + vs + 128],
                                    mask_sb[:],
                                    mult,
                                )
                        for j in (0, 1):
                            ki = 2 * blk + j
                            nc.tensor.matmul(
                                yq[:],
                                Vt[ki][:, 65 * hl : 65 * hl + 65],
                                es[:, 512 * j : 512 * j + 512],
                                start=(ki == 0),
                                stop=(ki == nki - 1),
                                skip_group_check=True,
                            )
                    rc = rcpool.tile([1, 512], F32, name="rc", tag="rc")
                    nc.vector.reciprocal(rc[:], yq[64:65, :])
                    rcb = rcpool.tile([64, 512], F32, name="rcb", tag="rcb")
                    nc.gpsimd.partition_broadcast(rcb[:], rc[:])
                    nc.vector.tensor_tensor(
                        yT[p][pr : pr + 64, 512 * qc : 512 * qc + 512],
                        yq[0:64, :],
                        rcb[:],
                        mult,
                    )

        # ---- phase 4: out-projection (row-sharded partial) ----
        with (
            tc.tile_pool(name="wpj", bufs=1) as wppool,
            tc.tile_pool(name="osb", bufs=3) as opool,
            tc.tile_pool(name="pso", bufs=4, space="PSUM") as pso,
        ):
            wp_sb = wppool.tile([128, 4, 1024], F32R, name="wp", tag="wp")
            dma(out=wp_sb[:], in_=d["wproj"].ap().rearrange("(pp p) n -> p pp n", p=128))
            for tt in range(TT):
                for cc in range(2):
                    ps = pso.tile([128, 512], F32, name="pso", tag="pso")
                    for pp in range(4):
                        nc.tensor.matmul(
                            ps[:],
                            yT[pp][:, 128 * tt : 128 * tt + 128],
                            wp_sb[:, pp, 512 * cc : 512 * cc + 512],
                            start=(pp == 0),
                            stop=(pp == 3),
                        )
                    ob = opool.tile([128, 512], F32, name="ob", tag="ob")
                    nc.scalar.copy(ob[:], ps[:])
                    dma(
                        out=d["out"].ap()[
                            128 * tt : 128 * tt + 128, 512 * cc : 512 * cc + 512
                        ],
                        in_=ob[:],
                    )


def _build():
    nc = bacc.Bacc("TRN2", target_bir_lowering=False, debug=False, num_devices=N_CORES)
    d = {
        "xT": nc.dram_tensor("xT", [C, T], F32R, kind="ExternalInput"),
        "wq": nc.dram_tensor("wq", [C, 512], F32R, kind="ExternalInput"),
        "wk": nc.dram_tensor("wk", [C, 512], F32R, kind="ExternalInput"),
        "wv": nc.dram_tensor("wv", [C, 520], F32R, kind="ExternalInput"),
        "bv": nc.dram_tensor("bv", [128, 520], F32, kind="ExternalInput"),
        "bq": nc.dram_tensor("bq", [128, 4], F32, kind="ExternalInput"),
        "bk": nc.dram_tensor("bk", [128, 4], F32, kind="ExternalInput"),
        "mask": nc.dram_tensor("mask", [128, 128], F32R, kind="ExternalInput"),
        "wproj": nc.dram_tensor("wproj", [512, 1024], F32R, kind="ExternalInput"),
        "out": nc.dram_tensor("out", [T, C], F32, kind="ExternalOutput"),
    }
    with tile.TileContext(nc) as tcx:
        _emit(nc, tcx, d)
    nc.compile()
    return nc


def _prep_core_inputs(c, x, w_attn, b_attn):
    g = c % 2
    xT = np.ascontiguousarray(x[c // 2].T)
    wq = np.ascontiguousarray(w_attn[:, 512 * g : 512 * g + 512])
    wk = np.ascontiguousarray(w_attn[:, 1024 + 512 * g : 1024 + 512 * g + 512])
    wv = np.zeros((C, 520), np.float32)
    bv = np.zeros((128, 520), np.float32)
    for hl in range(8):
        hcol = 2048 + 512 * g + 64 * hl
        wv[:, 65 * hl : 65 * hl + 64] = w_attn[:, hcol : hcol + 64]
        bv[:, 65 * hl : 65 * hl + 64] = b_attn[hcol : hcol + 64][None, :]
        bv[:, 65 * hl + 64] = 1.0
    bq = np.zeros((128, 4), np.float32)
    bk = np.zeros((128, 4), np.float32)
    for p in range(4):
        bq[:, p] = b_attn[512 * g + 128 * p : 512 * g + 128 * p + 128] * 0.125
        bk[:, p] = b_attn[1024 + 512 * g + 128 * p : 1024 + 512 * g + 128 * p + 128]
    mask = (np.arange(128)[:, None] <= np.arange(128)[None, :]).astype(np.float32)
    return dict(xT=xT, wq=wq, wk=wk, wv=wv, bv=bv, bq=bq, bk=bk, mask=mask)


def make_in_maps(x, w_attn, b_attn, w_proj):
    x = np.asarray(x, np.float32)
    w_attn = np.asarray(w_attn, np.float32)
    b_attn = np.asarray(b_attn, np.float32)
    w_proj = np.asarray(w_proj, np.float32)
    in_maps = []
    for c in range(N_CORES):
        m = _prep_core_inputs(c, x, w_attn, b_attn)
        g = c % 2
        m["wproj"] = np.ascontiguousarray(w_proj[512 * g : 512 * g + 512, :])
        in_maps.append(m)
    return in_maps


def get_nc():
    if "nc" not in _cache:
        _cache["nc"] = _build()
    return _cache["nc"]


def gather(results, b_proj):
    b_proj = np.asarray(b_proj, np.float32)
    full = np.empty((B, T, C), np.float32)
    for b in range(B):
        full[b] = results[2 * b]["out"] + results[2 * b + 1]["out"] + b_proj[None, :]
    return full


def kernel(x, w_attn, b_attn, w_proj, b_proj):
    nc = get_nc()
    in_maps = make_in_maps(x, w_attn, b_attn, w_proj)
    res = run_bass_kernel_spmd(nc, in_maps, list(range(N_CORES)))
    return gather(res.results, b_proj)


# revision 11
# speedup vs baseline: 2.0380x; 1.0028x over previous
"""Causal self-attention (B=4, T=2048, C=1024, 16 heads) on 8 trn2 NeuronCores.

Sharding: core c = (batch c//2, head-group c%2 of 8 heads). Data-parallel over
batch, tensor-parallel over heads; out-proj is row-sharded and the two partial
products per batch are summed on the host (no device collectives).

Device program per core (all fp32):
  phase 1: q^T/k^T = W^T @ x^T as head-pair tiles [128, T] (d on partitions)
  phase 2: V in natural [t, d] layout, augmented with a ones column per head
           (bias trick) so P@V also accumulates softmax row-sums for free
  phase 3: flash-style attention in S^T layout (S computed transposed — no PE
           transposes, no max subtraction: |S| < ~3 by construction), causal
           masking via a single static 128x128 triangular mask + memsets,
           normalization folded into the PSUM->SBUF copy
  phase 4: out = y^T-slices^T @ W_proj rows (partial over this core's heads)
"""

import os
import sys

import numpy as np

for _p in ("/opt/trn_rl_repo", "/root/.axon_site/_ro/trn_rl_repo"):
    if os.path.isdir(_p) and _p not in sys.path:
        sys.path.insert(0, _p)

import concourse.bass as bass  # noqa: E402
import concourse.tile as tile  # noqa: E402
from concourse import bacc, mybir  # noqa: E402
from concourse.bass_utils import run_bass_kernel_spmd  # noqa: E402

B, T, C = 4, 2048, 1024
H, D = 16, 64
N_CORES = 8
F32 = mybir.dt.float32
F32R = mybir.dt.float32r


def _r(ap):
    return ap.bitcast(F32R)
TC = T // 512  # 4 t-chunks of 512
TT = T // 128  # 16 t-tiles of 128
CT = C // 128  # 8 c-tiles of 128

_cache: dict = {}


def _emit(nc: "bacc.Bacc", tc: "tile.TileContext", d: dict) -> None:
    mult = mybir.AluOpType.mult
    add = mybir.AluOpType.add
    Exp = mybir.ActivationFunctionType.Exp
    dma = nc.sync.dma_start

    with (
        tc.tile_pool(name="const", bufs=1) as cpool,
        tc.tile_pool(name="persist", bufs=1) as persist,
    ):
        madd_sb = cpool.tile([128, 2, 1024], F32, name="madd", tag="madd")
        dma(out=madd_sb[:], in_=d["madd"].ap())
        bq_sb = cpool.tile([128, 4], F32, name="bq", tag="bq")
        dma(out=bq_sb[:], in_=d["bq"].ap())
        bk_sb = cpool.tile([128, 4], F32, name="bk", tag="bk")
        dma(out=bk_sb[:], in_=d["bk"].ap())
        bv_sb = cpool.tile([128, 520], F32, name="bv", tag="bv")
        dma(out=bv_sb[:], in_=d["bv"].ap())

        qT = [persist.tile([128, T], F32R, name=f"qT{p}", tag=f"qT{p}") for p in range(4)]
        kT = [persist.tile([128, T], F32R, name=f"kT{p}", tag=f"kT{p}") for p in range(4)]
        Vt = [persist.tile([128, 520], F32R, name=f"V{i}", tag=f"V{i}") for i in range(TT)]
        yT = [persist.tile([128, T], F32R, name=f"yT{p}", tag=f"yT{p}") for p in range(4)]

        # ---- phase 1: q^T / k^T head-pair tiles ----
        with (
            tc.tile_pool(name="wqk", bufs=1) as wpool,
            tc.tile_pool(name="xt1", bufs=2) as xpool,
            tc.tile_pool(name="psqk", bufs=4, space="PSUM") as pq,
        ):
            wq_sb = wpool.tile([128, CT, 512], F32R, name="wq", tag="wq")
            dma(out=wq_sb[:], in_=d["wq"].ap().rearrange("(ct p) n -> p ct n", p=128))
            wk_sb = wpool.tile([128, CT, 512], F32R, name="wk", tag="wk")
            dma(out=wk_sb[:], in_=d["wk"].ap().rearrange("(ct p) n -> p ct n", p=128))
            for tci in range(TC):
                xt = xpool.tile([128, CT, 512], F32R, name="xt", tag="xt")
                dma(
                    out=xt[:],
                    in_=d["xT"].ap()[:, 512 * tci : 512 * tci + 512].rearrange(
                        "(ct p) n -> p ct n", p=128
                    ),
                )
                for p in range(4):
                    for w_sb, dstT, scale, bias_sb in (
                        (wq_sb, qT, 0.125, bq_sb),
                        (wk_sb, kT, 1.0, bk_sb),
                    ):
                        ps = pq.tile([128, 512], F32, name="psqk", tag="psqk")
                        for ct in range(CT):
                            nc.tensor.matmul(
                                ps[:],
                                w_sb[:, ct, 128 * p : 128 * p + 128],
                                xt[:, ct, :],
                                start=(ct == 0),
                                stop=(ct == CT - 1),
                            )
                        nc.vector.tensor_scalar(
                            dstT[p][:, 512 * tci : 512 * tci + 512],
                            ps[:],
                            scale,
                            bias_sb[:, p : p + 1],
                            mult,
                            add,
                        )

        # ---- phase 2: V (natural layout, ones-augmented) ----
        with (
            tc.tile_pool(name="wv", bufs=1) as wvpool,
            tc.tile_pool(name="xt2", bufs=2) as xpool2,
            tc.tile_pool(name="psv", bufs=4, space="PSUM") as pv,
        ):
            wv_sb = wvpool.tile([128, CT, 520], F32R, name="wv", tag="wv")
            dma(out=wv_sb[:], in_=d["wv"].ap().rearrange("(ct p) n -> p ct n", p=128))
            for tci in range(TC):
                xt = xpool2.tile([128, CT, 512], F32R, name="xt2", tag="xt2")
                dma(
                    out=xt[:],
                    in_=d["xT"].ap()[:, 512 * tci : 512 * tci + 512].rearrange(
                        "(ct p) n -> p ct n", p=128
                    ),
                )
                for tt in range(4):
                    for qd in range(2):
                        ps = pv.tile([128, 260], F32, name="psv", tag="psv")
                        for ct in range(CT):
                            nc.tensor.matmul(
                                ps[:],
                                xt[:, ct, 128 * tt : 128 * tt + 128],
                                wv_sb[:, ct, 260 * qd : 260 * qd + 260],
                                start=(ct == 0),
                                stop=(ct == CT - 1),
                            )
                        nc.vector.tensor_tensor(
                            Vt[4 * tci + tt][:, 260 * qd : 260 * qd + 260],
                            ps[:],
                            bv_sb[:, 260 * qd : 260 * qd + 260],
                            add,
                        )

        wppool = tc.alloc_tile_pool(name="wpj", bufs=1)
        wp_sb = wppool.tile([128, 4, 1024], F32R, name="wp", tag="wp")
        dma(out=wp_sb[:], in_=d["wproj"].ap().rearrange("(pp p) n -> p pp n", p=128))

        # ---- phase 3: attention, S^T layout ----
        with (
            tc.tile_pool(name="es", bufs=3) as espool,
            tc.tile_pool(name="rc", bufs=2) as rcpool,
            tc.tile_pool(name="pss", bufs=3, space="PSUM") as pss,
            tc.tile_pool(name="psy", bufs=2, space="PSUM") as psy,
        ):
            for hl in range(8):
                p, half = hl // 2, hl % 2
                pr = 64 * half
                rsum = rcpool.tile([1, T], F32, name="rsum", tag="rsum")
                for qc in range(4):
                    yq = psy.tile([65, 512], F32, name="yq", tag="yq")
                    nki = 4 * qc + 4
                    for blk in range(nki // 2):
                        sblk = pss.tile([128, 1024], F32, name="sblk", tag="sblk")
                        for j in (0, 1):
                            ki = 2 * blk + j
                            sc = 512 * j
                            nc.tensor.matmul(
                                sblk[:, sc : sc + 512],
                                kT[p][pr : pr + 64, 128 * ki : 128 * ki + 128],
                                qT[p][pr : pr + 64, 512 * qc : 512 * qc + 512],
                                start=True,
                                stop=True,
                            )
                        dblk = blk - 2 * qc
                        if dblk >= 0:
                            nc.vector.tensor_tensor(
                                sblk[:, :],
                                sblk[:, :],
                                madd_sb[:, dblk, :],
                                add,
                            )
                        es = espool.tile([128, 1024], F32R, name="es", tag="es")
                        nc.scalar.activation(es[:], sblk[:], Exp)
                        for j in (0, 1):
                            ki = 2 * blk + j
                            nc.tensor.matmul(
                                yq[:],
                                Vt[ki][:, 65 * hl : 65 * hl + 65],
                                es[:, 512 * j : 512 * j + 512],
                                start=(ki == 0),
                                stop=(ki == nki - 1),
                                skip_group_check=True,
                            )
                    nc.vector.tensor_copy(
                        out=yT[p][pr : pr + 64, 512 * qc : 512 * qc + 512],
                        in_=yq[0:64, :],
                    )
                    nc.vector.tensor_copy(
                        out=rsum[:, 512 * qc : 512 * qc + 512], in_=yq[64:65, :]
                    )
                rs8 = rcpool.tile([64, 32], F32, name="rs8", tag="rs8")
                dma(out=rs8[:], in_=rsum[:])
                rr8 = rcpool.tile([64, 32], F32, name="rr8", tag="rr8")
                nc.vector.reciprocal(rr8[:], rs8[:])
                dma(out=rsum[:], in_=rr8[:])
                rcb = rcpool.tile([128, T], F32, name="rcb", tag="rcb")
                nc.gpsimd.partition_broadcast(rcb[:], rsum[:])
                nc.vector.tensor_tensor(
                    yT[p][pr : pr + 64, :],
                    yT[p][pr : pr + 64, :],
                    rcb[pr : pr + 64, :],
                    mult,
                )

        # ---- phase 4: out-projection (row-sharded partial) ----
        with (
            tc.tile_pool(name="osb", bufs=3) as opool,
            tc.tile_pool(name="pso", bufs=4, space="PSUM") as pso,
        ):
            for tt in range(TT):
                for cc in range(2):
                    ps = pso.tile([128, 512], F32, name="pso", tag="pso")
                    for pp in range(4):
                        nc.tensor.matmul(
                            ps[:],
                            yT[pp][:, 128 * tt : 128 * tt + 128],
                            wp_sb[:, pp, 512 * cc : 512 * cc + 512],
                            start=(pp == 0),
                            stop=(pp == 3),
                        )
                    ob = opool.tile([128, 512], F32, name="ob", tag="ob")
                    nc.scalar.copy(ob[:], ps[:])
                    dma(
                        out=d["out"].ap()[
                            128 * tt : 128 * tt + 128, 512 * cc : 512 * cc + 512
                        ],
                        in_=ob[:],
                    )
        wppool.release()


def _build():
    nc = bacc.Bacc("TRN2", target_bir_lowering=False, debug=False, num_devices=N_CORES)
    d = {
        "xT": nc.dram_tensor("xT", [C, T], F32R, kind="ExternalInput"),
        "wq": nc.dram_tensor("wq", [C, 512], F32R, kind="ExternalInput"),
        "wk": nc.dram_tensor("wk", [C, 512], F32R, kind="ExternalInput"),
        "wv": nc.dram_tensor("wv", [C, 520], F32R, kind="ExternalInput"),
        "bv": nc.dram_tensor("bv", [128, 520], F32, kind="ExternalInput"),
        "bq": nc.dram_tensor("bq", [128, 4], F32, kind="ExternalInput"),
        "bk": nc.dram_tensor("bk", [128, 4], F32, kind="ExternalInput"),
        "madd": nc.dram_tensor("madd", [128, 2, 1024], F32, kind="ExternalInput"),
        "wproj": nc.dram_tensor("wproj", [512, 1024], F32R, kind="ExternalInput"),
        "out": nc.dram_tensor("out", [T, C], F32, kind="ExternalOutput"),
    }
    with tile.TileContext(nc) as tcx:
        _emit(nc, tcx, d)
    nc.compile()
    return nc


def _prep_core_inputs(c, x, w_attn, b_attn):
    g = c % 2
    xT = np.ascontiguousarray(x[c // 2].T)
    wq = np.ascontiguousarray(w_attn[:, 512 * g : 512 * g + 512])
    wk = np.ascontiguousarray(w_attn[:, 1024 + 512 * g : 1024 + 512 * g + 512])
    wv = np.zeros((C, 520), np.float32)
    bv = np.zeros((128, 520), np.float32)
    for hl in range(8):
        hcol = 2048 + 512 * g + 64 * hl
        wv[:, 65 * hl : 65 * hl + 64] = w_attn[:, hcol : hcol + 64]
        bv[:, 65 * hl : 65 * hl + 64] = b_attn[hcol : hcol + 64][None, :]
        bv[:, 65 * hl + 64] = 1.0
    bq = np.zeros((128, 4), np.float32)
    bk = np.zeros((128, 4), np.float32)
    for p in range(4):
        bq[:, p] = b_attn[512 * g + 128 * p : 512 * g + 128 * p + 128] * 0.125
        bk[:, p] = b_attn[1024 + 512 * g + 128 * p : 1024 + 512 * g + 128 * p + 128]
    madd = np.zeros((128, 2, 1024), np.float32)
    pp = np.arange(128)[:, None]
    jj = np.arange(512)[None, :]
    for r in range(4):
        madd[:, r // 2, 512 * (r % 2) : 512 * (r % 2) + 512] = np.where(
            jj >= 128 * r + pp, 0.0, -60.0
        )
    return dict(xT=xT, wq=wq, wk=wk, wv=wv, bv=bv, bq=bq, bk=bk, madd=madd)


def make_in_maps(x, w_attn, b_attn, w_proj):
    x = np.asarray(x, np.float32)
    w_attn = np.asarray(w_attn, np.float32)
    b_attn = np.asarray(b_attn, np.float32)
    w_proj = np.asarray(w_proj, np.float32)
    in_maps = []
    for c in range(N_CORES):
        m = _prep_core_inputs(c, x, w_attn, b_attn)
        g = c % 2
        m["wproj"] = np.ascontiguousarray(w_proj[512 * g : 512 * g + 512, :])
        in_maps.append(m)
    return in_maps


def get_nc():
    if "nc" not in _cache:
        _cache["nc"] = _build()
    return _cache["nc"]


def gather(results, b_proj):
    b_proj = np.asarray(b_proj, np.float32)
    full = np.empty((B, T, C), np.float32)
    for b in range(B):
        full[b] = results[2 * b]["out"] + results[2 * b + 1]["out"] + b_proj[None, :]
    return full


def kernel(x, w_attn, b_attn, w_proj, b_proj):
    nc = get_nc()
    in_maps = make_in_maps(x, w_attn, b_attn, w_proj)
    res = run_bass_kernel_spmd(nc, in_maps, list(range(N_CORES)))
    return gather(res.results, b_proj)


# revision 16
# speedup vs baseline: 2.5487x; 1.2506x over previous
"""Causal self-attention (B=4, T=2048, C=1024, 16 heads) on 8 trn2 NeuronCores.

Sharding: core c = (batch c//2, head-group c%2 of 8 heads). Data-parallel over
batch, tensor-parallel over heads; out-proj is row-sharded and the two partial
products per batch are summed on the host (no device collectives).

Device program per core (all fp32):
  phase 1: q^T/k^T = W^T @ x^T as head-pair tiles [128, T] (d on partitions)
  phase 2: V in natural [t, d] layout, augmented with a ones column per head
           (bias trick) so P@V also accumulates softmax row-sums for free
  phase 3: flash-style attention in S^T layout (S computed transposed — no PE
           transposes, no max subtraction: |S| < ~3 by construction), causal
           masking via a single static 128x128 triangular mask + memsets,
           normalization folded into the PSUM->SBUF copy
  phase 4: out = y^T-slices^T @ W_proj rows (partial over this core's heads)
"""

import os
import sys

import numpy as np

for _p in ("/opt/trn_rl_repo", "/root/.axon_site/_ro/trn_rl_repo"):
    if os.path.isdir(_p) and _p not in sys.path:
        sys.path.insert(0, _p)

import concourse.bass as bass  # noqa: E402
import concourse.tile as tile  # noqa: E402
from concourse import bacc, mybir  # noqa: E402
from concourse.bass_utils import run_bass_kernel_spmd  # noqa: E402

B, T, C = 4, 2048, 1024
H, D = 16, 64
N_CORES = 8
F32 = mybir.dt.float32
F32R = mybir.dt.float32r
BF16 = mybir.dt.bfloat16


def _r(ap):
    return ap.bitcast(F32R)
TC = T // 512  # 4 t-chunks of 512
TT = T // 128  # 16 t-tiles of 128
CT = C // 128  # 8 c-tiles of 128

_cache: dict = {}


def _emit(nc: "bacc.Bacc", tc: "tile.TileContext", d: dict) -> None:
    mult = mybir.AluOpType.mult
    add = mybir.AluOpType.add
    Exp = mybir.ActivationFunctionType.Exp
    dma = nc.sync.dma_start

    with (
        tc.tile_pool(name="const", bufs=1) as cpool,
        tc.tile_pool(name="persist", bufs=1) as persist,
    ):
        madd_sb = cpool.tile([128, 4, 512], F32, name="madd", tag="madd")
        dma(out=madd_sb[:], in_=d["madd"].ap())
        bq_sb = cpool.tile([128, 4], F32, name="bq", tag="bq")
        dma(out=bq_sb[:], in_=d["bq"].ap())
        bk_sb = cpool.tile([128, 4], F32, name="bk", tag="bk")
        dma(out=bk_sb[:], in_=d["bk"].ap())
        bv_sb = cpool.tile([128, 520], F32, name="bv", tag="bv")
        dma(out=bv_sb[:], in_=d["bv"].ap())

        qT = [persist.tile([128, T], BF16, name=f"qT{p}", tag=f"qT{p}") for p in range(4)]
        kTp = [
            [
                persist.tile([128, T], BF16, name=f"kT{p}_{h2}", tag=f"kT{p}_{h2}")
                for h2 in (0, 1)
            ]
            for p in range(4)
        ]
        Vt = [persist.tile([128, 520], BF16, name=f"V{i}", tag=f"V{i}") for i in range(TT)]
        yT = [persist.tile([128, T], F32R, name=f"yT{p}", tag=f"yT{p}") for p in range(4)]
        for p in range(4):
            nc.gpsimd.memset(kTp[p][0][64:128, :], 0.0)
            nc.gpsimd.memset(kTp[p][1][0:64, :], 0.0)

        # ---- phase 1: q^T / k^T head-pair tiles ----
        with (
            tc.tile_pool(name="wqk", bufs=1) as wpool,
            tc.tile_pool(name="xt1", bufs=2) as xpool,
            tc.tile_pool(name="psqk", bufs=4, space="PSUM") as pq,
        ):
            wq_sb = wpool.tile([128, CT, 512], F32R, name="wq", tag="wq")
            dma(out=wq_sb[:], in_=d["wq"].ap().rearrange("(ct p) n -> p ct n", p=128))
            wk_sb = wpool.tile([128, CT, 512], F32R, name="wk", tag="wk")
            dma(out=wk_sb[:], in_=d["wk"].ap().rearrange("(ct p) n -> p ct n", p=128))
            for tci in range(TC):
                xt = xpool.tile([128, CT, 512], F32R, name="xt", tag="xt")
                dma(
                    out=xt[:],
                    in_=d["xT"].ap()[:, 512 * tci : 512 * tci + 512].rearrange(
                        "(ct p) n -> p ct n", p=128
                    ),
                )
                for p in range(4):
                    for iw, w_sb in ((0, wq_sb), (1, wk_sb)):
                        ps = pq.tile([128, 512], F32, name="psqk", tag="psqk")
                        for ct in range(CT):
                            nc.tensor.matmul(
                                ps[:],
                                w_sb[:, ct, 128 * p : 128 * p + 128],
                                xt[:, ct, :],
                                start=(ct == 0),
                                stop=(ct == CT - 1),
                            )
                        if iw == 0:
                            nc.vector.tensor_scalar(
                                qT[p][:, 512 * tci : 512 * tci + 512],
                                ps[:],
                                0.125,
                                bq_sb[:, p : p + 1],
                                mult,
                                add,
                            )
                        else:
                            for h2 in (0, 1):
                                pr = 64 * h2
                                nc.vector.tensor_scalar(
                                    kTp[p][h2][
                                        pr : pr + 64, 512 * tci : 512 * tci + 512
                                    ],
                                    ps[pr : pr + 64, :],
                                    1.0,
                                    bk_sb[pr : pr + 64, p : p + 1],
                                    mult,
                                    add,
                                )

        # ---- phase 2: V (natural layout, ones-augmented) ----
        with (
            tc.tile_pool(name="wv", bufs=1) as wvpool,
            tc.tile_pool(name="xt2", bufs=2) as xpool2,
            tc.tile_pool(name="psv", bufs=4, space="PSUM") as pv,
        ):
            wv_sb = wvpool.tile([128, CT, 520], F32R, name="wv", tag="wv")
            dma(out=wv_sb[:], in_=d["wv"].ap().rearrange("(ct p) n -> p ct n", p=128))
            for tci in range(TC):
                xt = xpool2.tile([128, CT, 512], F32R, name="xt2", tag="xt2")
                dma(
                    out=xt[:],
                    in_=d["xT"].ap()[:, 512 * tci : 512 * tci + 512].rearrange(
                        "(ct p) n -> p ct n", p=128
                    ),
                )
                for tt in range(4):
                    for qd in range(2):
                        ps = pv.tile([128, 260], F32, name="psv", tag="psv")
                        for ct in range(CT):
                            nc.tensor.matmul(
                                ps[:],
                                xt[:, ct, 128 * tt : 128 * tt + 128],
                                wv_sb[:, ct, 260 * qd : 260 * qd + 260],
                                start=(ct == 0),
                                stop=(ct == CT - 1),
                            )
                        nc.vector.tensor_tensor(
                            Vt[4 * tci + tt][:, 260 * qd : 260 * qd + 260],
                            ps[:],
                            bv_sb[:, 260 * qd : 260 * qd + 260],
                            add,
                        )

        wppool = tc.alloc_tile_pool(name="wpj", bufs=1)
        wp_sb = wppool.tile([128, 4, 1024], F32R, name="wp", tag="wp")
        dma(out=wp_sb[:], in_=d["wproj"].ap().rearrange("(pp p) n -> p pp n", p=128))

        # ---- phase 3: attention in S^T layout, head pairs, K=128 via padded kT ----
        with (
            tc.tile_pool(name="es", bufs=3) as espool,
            tc.tile_pool(name="rc", bufs=2) as rcpool,
            tc.tile_pool(name="pss", bufs=2, space="PSUM") as pss,
            tc.tile_pool(name="psy", bufs=2, space="PSUM") as psy,
        ):
            for p in range(4):
                rsums = [
                    rcpool.tile([1, T], F32, name=f"rsum{h2}", tag=f"rsum{h2}")
                    for h2 in (0, 1)
                ]
                for qc in range(4):
                    yqs = [
                        psy.tile([65, 512], F32, name=f"yq{h2}", tag=f"yq{h2}")
                        for h2 in (0, 1)
                    ]
                    nki = 4 * qc + 4
                    for ki in range(nki):
                        sblk = pss.tile([128, 1024], F32, name="sblk", tag="sblk")
                        for h2 in (0, 1):
                            nc.tensor.matmul(
                                sblk[:, 512 * h2 : 512 * h2 + 512],
                                kTp[p][h2][:, 128 * ki : 128 * ki + 128],
                                qT[p][:, 512 * qc : 512 * qc + 512],
                                start=True,
                                stop=True,
                            )
                        r = ki - 4 * qc
                        if r >= 0:
                            w = 128 * (r + 1)
                            s2 = sblk.rearrange("q (s f) -> q s f", s=2)
                            m1 = madd_sb[:, r, 0:w]
                            m2 = bass.AP(
                                tensor=m1.tensor,
                                offset=m1.offset,
                                ap=[list(m1.ap[0]), [0, 2], list(m1.ap[1])],
                            )
                            nc.vector.tensor_tensor(
                                s2[:, :, 0:w], s2[:, :, 0:w], m2, add
                            )
                        es = espool.tile([128, 1024], BF16, name="es", tag="es")
                        nc.scalar.activation(es[:], sblk[:], Exp)
                        first, last = ki == 0, ki == nki - 1
                        for h2 in (0, 1):
                            hl = 2 * p + h2
                            nc.tensor.matmul(
                                yqs[h2][:],
                                Vt[ki][:, 65 * hl : 65 * hl + 65],
                                es[:, 512 * h2 : 512 * h2 + 512],
                                start=first,
                                stop=last,
                                skip_group_check=True,
                            )
                    for h2 in (0, 1):
                        pr = 64 * h2
                        nc.vector.tensor_copy(
                            out=yT[p][pr : pr + 64, 512 * qc : 512 * qc + 512],
                            in_=yqs[h2][0:64, :],
                        )
                        nc.vector.tensor_copy(
                            out=rsums[h2][:, 512 * qc : 512 * qc + 512],
                            in_=yqs[h2][64:65, :],
                        )
                for h2 in (0, 1):
                    pr = 64 * h2
                    rs8 = rcpool.tile([64, 32], F32, name="rs8", tag="rs8")
                    dma(out=rs8[:], in_=rsums[h2][:])
                    rr8 = rcpool.tile([64, 32], F32, name="rr8", tag="rr8")
                    nc.vector.reciprocal(rr8[:], rs8[:])
                    dma(out=rsums[h2][:], in_=rr8[:])
                    rcb = rcpool.tile([128, T], F32, name="rcb", tag="rcb", bufs=1)
                    nc.gpsimd.partition_broadcast(rcb[:], rsums[h2][:])
                    nc.vector.tensor_tensor(
                        yT[p][pr : pr + 64, :],
                        yT[p][pr : pr + 64, :],
                        rcb[pr : pr + 64, :],
                        mult,
                    )

        # ---- phase 4: out-projection (row-sharded partial) ----
        with (
            tc.tile_pool(name="osb", bufs=3) as opool,
            tc.tile_pool(name="pso", bufs=4, space="PSUM") as pso,
        ):
            for tt in range(TT):
                for cc in range(2):
                    ps = pso.tile([128, 512], F32, name="pso", tag="pso")
                    for pp in range(4):
                        nc.tensor.matmul(
                            ps[:],
                            yT[pp][:, 128 * tt : 128 * tt + 128],
                            wp_sb[:, pp, 512 * cc : 512 * cc + 512],
                            start=(pp == 0),
                            stop=(pp == 3),
                        )
                    ob = opool.tile([128, 512], F32, name="ob", tag="ob")
                    nc.scalar.copy(ob[:], ps[:])
                    dma(
                        out=d["out"].ap()[
                            128 * tt : 128 * tt + 128, 512 * cc : 512 * cc + 512
                        ],
                        in_=ob[:],
                    )
        wppool.release()


def _build():
    nc = bacc.Bacc("TRN2", target_bir_lowering=False, debug=False, num_devices=N_CORES)
    d = {
        "xT": nc.dram_tensor("xT", [C, T], F32R, kind="ExternalInput"),
        "wq": nc.dram_tensor("wq", [C, 512], F32R, kind="ExternalInput"),
        "wk": nc.dram_tensor("wk", [C, 512], F32R, kind="ExternalInput"),
        "wv": nc.dram_tensor("wv", [C, 520], F32R, kind="ExternalInput"),
        "bv": nc.dram_tensor("bv", [128, 520], F32, kind="ExternalInput"),
        "bq": nc.dram_tensor("bq", [128, 4], F32, kind="ExternalInput"),
        "bk": nc.dram_tensor("bk", [128, 4], F32, kind="ExternalInput"),
        "madd": nc.dram_tensor("madd", [128, 4, 512], F32, kind="ExternalInput"),
        "wproj": nc.dram_tensor("wproj", [512, 1024], F32R, kind="ExternalInput"),
        "out": nc.dram_tensor("out", [T, C], F32, kind="ExternalOutput"),
    }
    with tile.TileContext(nc) as tcx:
        _emit(nc, tcx, d)
    nc.compile()
    return nc


def _prep_core_inputs(c, x, w_attn, b_attn):
    g = c % 2
    xT = np.ascontiguousarray(x[c // 2].T)
    wq = np.ascontiguousarray(w_attn[:, 512 * g : 512 * g + 512])
    wk = np.ascontiguousarray(w_attn[:, 1024 + 512 * g : 1024 + 512 * g + 512])
    wv = np.zeros((C, 520), np.float32)
    bv = np.zeros((128, 520), np.float32)
    for hl in range(8):
        hcol = 2048 + 512 * g + 64 * hl
        wv[:, 65 * hl : 65 * hl + 64] = w_attn[:, hcol : hcol + 64]
        bv[:, 65 * hl : 65 * hl + 64] = b_attn[hcol : hcol + 64][None, :]
        bv[:, 65 * hl + 64] = 1.0
    bq = np.zeros((128, 4), np.float32)
    bk = np.zeros((128, 4), np.float32)
    for p in range(4):
        bq[:, p] = b_attn[512 * g + 128 * p : 512 * g + 128 * p + 128] * 0.125
        bk[:, p] = b_attn[1024 + 512 * g + 128 * p : 1024 + 512 * g + 128 * p + 128]
    madd = np.zeros((128, 4, 512), np.float32)
    pp = np.arange(128)[:, None]
    jj = np.arange(512)[None, :]
    for r in range(4):
        madd[:, r, :] = np.where(jj >= 128 * r + pp, 0.0, -60.0)
    return dict(xT=xT, wq=wq, wk=wk, wv=wv, bv=bv, bq=bq, bk=bk, madd=madd)


def make_in_maps(x, w_attn, b_attn, w_proj):
    x = np.asarray(x, np.float32)
    w_attn = np.asarray(w_attn, np.float32)
    b_attn = np.asarray(b_attn, np.float32)
    w_proj = np.asarray(w_proj, np.float32)
    in_maps = []
    for c in range(N_CORES):
        m = _prep_core_inputs(c, x, w_attn, b_attn)
        g = c % 2
        m["wproj"] = np.ascontiguousarray(w_proj[512 * g : 512 * g + 512, :])
        in_maps.append(m)
    return in_maps


def get_nc():
    if "nc" not in _cache:
        _cache["nc"] = _build()
    return _cache["nc"]


def gather(results, b_proj):
    b_proj = np.asarray(b_proj, np.float32)
    full = np.empty((B, T, C), np.float32)
    for b in range(B):
        full[b] = results[2 * b]["out"] + results[2 * b + 1]["out"] + b_proj[None, :]
    return full


def kernel(x, w_attn, b_attn, w_proj, b_proj):
    nc = get_nc()
    in_maps = make_in_maps(x, w_attn, b_attn, w_proj)
    res = run_bass_kernel_spmd(nc, in_maps, list(range(N_CORES)))
    return gather(res.results, b_proj)


# revision 17
# speedup vs baseline: 3.0553x; 1.1988x over previous
"""Causal self-attention (B=4, T=2048, C=1024, 16 heads) on 8 trn2 NeuronCores.

Sharding: core c = (batch c//2, head-group c%2 of 8 heads). Data-parallel over
batch, tensor-parallel over heads; out-proj is row-sharded and the two partial
products per batch are summed on the host (no device collectives).

Device program per core (all fp32):
  phase 1: q^T/k^T = W^T @ x^T as head-pair tiles [128, T] (d on partitions)
  phase 2: V in natural [t, d] layout, augmented with a ones column per head
           (bias trick) so P@V also accumulates softmax row-sums for free
  phase 3: flash-style attention in S^T layout (S computed transposed — no PE
           transposes, no max subtraction: |S| < ~3 by construction), causal
           masking via a single static 128x128 triangular mask + memsets,
           normalization folded into the PSUM->SBUF copy
  phase 4: out = y^T-slices^T @ W_proj rows (partial over this core's heads)
"""

import os
import sys

import numpy as np

for _p in ("/opt/trn_rl_repo", "/root/.axon_site/_ro/trn_rl_repo"):
    if os.path.isdir(_p) and _p not in sys.path:
        sys.path.insert(0, _p)

import concourse.bass as bass  # noqa: E402
import concourse.tile as tile  # noqa: E402
from concourse import bacc, mybir  # noqa: E402
from concourse.bass_utils import run_bass_kernel_spmd  # noqa: E402

B, T, C = 4, 2048, 1024
H, D = 16, 64
N_CORES = 8
F32 = mybir.dt.float32
F32R = mybir.dt.float32r
BF16 = mybir.dt.bfloat16


def _r(ap):
    return ap.bitcast(F32R)
TC = T // 512  # 4 t-chunks of 512
TT = T // 128  # 16 t-tiles of 128
CT = C // 128  # 8 c-tiles of 128

_cache: dict = {}


def _emit(nc: "bacc.Bacc", tc: "tile.TileContext", d: dict) -> None:
    mult = mybir.AluOpType.mult
    add = mybir.AluOpType.add
    Exp = mybir.ActivationFunctionType.Exp
    dma = nc.sync.dma_start

    with (
        tc.tile_pool(name="const", bufs=1) as cpool,
        tc.tile_pool(name="persist", bufs=1) as persist,
    ):
        m01_sb = cpool.tile([128, 128], BF16, name="m01", tag="m01")
        dma(out=m01_sb[:], in_=d["m01"].ap())
        warm_sb = cpool.tile([128, 2], F32, name="warm", tag="warm")
        nc.vector.memset(warm_sb[:, 0:1], 0.0)
        nc.scalar.activation(
            warm_sb[:, 1:2], warm_sb[:, 0:1], mybir.ActivationFunctionType.Exp
        )
        bq_sb = cpool.tile([128, 4], F32, name="bq", tag="bq")
        dma(out=bq_sb[:], in_=d["bq"].ap())
        bk_sb = cpool.tile([128, 4], F32, name="bk", tag="bk")
        dma(out=bk_sb[:], in_=d["bk"].ap())
        bv_sb = cpool.tile([128, 520], F32, name="bv", tag="bv")
        dma(out=bv_sb[:], in_=d["bv"].ap())

        qT = [persist.tile([128, T], BF16, name=f"qT{p}", tag=f"qT{p}") for p in range(4)]
        kTp = [
            [
                persist.tile([128, T], BF16, name=f"kT{p}_{h2}", tag=f"kT{p}_{h2}")
                for h2 in (0, 1)
            ]
            for p in range(4)
        ]
        Vt = [persist.tile([128, 520], BF16, name=f"V{i}", tag=f"V{i}") for i in range(TT)]
        yT = [persist.tile([128, T], F32R, name=f"yT{p}", tag=f"yT{p}") for p in range(4)]
        for p in range(4):
            nc.gpsimd.memset(kTp[p][0][64:128, :], 0.0)
            nc.gpsimd.memset(kTp[p][1][0:64, :], 0.0)

        # ---- phase 1: q^T / k^T head-pair tiles ----
        with (
            tc.tile_pool(name="wqk", bufs=1) as wpool,
            tc.tile_pool(name="xt1", bufs=2) as xpool,
            tc.tile_pool(name="psqk", bufs=4, space="PSUM") as pq,
        ):
            wq_sb = wpool.tile([128, CT, 512], F32R, name="wq", tag="wq")
            dma(out=wq_sb[:], in_=d["wq"].ap().rearrange("(ct p) n -> p ct n", p=128))
            wk_sb = wpool.tile([128, CT, 512], F32R, name="wk", tag="wk")
            dma(out=wk_sb[:], in_=d["wk"].ap().rearrange("(ct p) n -> p ct n", p=128))
            for tci in range(TC):
                xt = xpool.tile([128, CT, 512], F32R, name="xt", tag="xt")
                dma(
                    out=xt[:],
                    in_=d["xT"].ap()[:, 512 * tci : 512 * tci + 512].rearrange(
                        "(ct p) n -> p ct n", p=128
                    ),
                )
                for p in range(4):
                    for iw, w_sb in ((0, wq_sb), (1, wk_sb)):
                        ps = pq.tile([128, 512], F32, name="psqk", tag="psqk")
                        for ct in range(CT):
                            nc.tensor.matmul(
                                ps[:],
                                w_sb[:, ct, 128 * p : 128 * p + 128],
                                xt[:, ct, :],
                                start=(ct == 0),
                                stop=(ct == CT - 1),
                            )
                        if iw == 0:
                            nc.vector.tensor_scalar(
                                qT[p][:, 512 * tci : 512 * tci + 512],
                                ps[:],
                                0.125,
                                bq_sb[:, p : p + 1],
                                mult,
                                add,
                            )
                        else:
                            for h2 in (0, 1):
                                pr = 64 * h2
                                nc.vector.tensor_scalar(
                                    kTp[p][h2][
                                        pr : pr + 64, 512 * tci : 512 * tci + 512
                                    ],
                                    ps[pr : pr + 64, :],
                                    1.0,
                                    bk_sb[pr : pr + 64, p : p + 1],
                                    mult,
                                    add,
                                )

        # ---- phase 2: V (natural layout, ones-augmented) ----
        with (
            tc.tile_pool(name="wv", bufs=1) as wvpool,
            tc.tile_pool(name="xt2", bufs=2) as xpool2,
            tc.tile_pool(name="psv", bufs=4, space="PSUM") as pv,
        ):
            wv_sb = wvpool.tile([128, CT, 520], F32R, name="wv", tag="wv")
            dma(out=wv_sb[:], in_=d["wv"].ap().rearrange("(ct p) n -> p ct n", p=128))
            for tci in range(TC):
                xt = xpool2.tile([128, CT, 512], F32R, name="xt2", tag="xt2")
                dma(
                    out=xt[:],
                    in_=d["xT"].ap()[:, 512 * tci : 512 * tci + 512].rearrange(
                        "(ct p) n -> p ct n", p=128
                    ),
                )
                for tt in range(4):
                    for qd in range(2):
                        ps = pv.tile([128, 260], F32, name="psv", tag="psv")
                        for ct in range(CT):
                            nc.tensor.matmul(
                                ps[:],
                                xt[:, ct, 128 * tt : 128 * tt + 128],
                                wv_sb[:, ct, 260 * qd : 260 * qd + 260],
                                start=(ct == 0),
                                stop=(ct == CT - 1),
                            )
                        nc.vector.tensor_tensor(
                            Vt[4 * tci + tt][:, 260 * qd : 260 * qd + 260],
                            ps[:],
                            bv_sb[:, 260 * qd : 260 * qd + 260],
                            add,
                        )

        wppool = tc.alloc_tile_pool(name="wpj", bufs=1)
        wp_sb = wppool.tile([128, 4, 1024], F32R, name="wp", tag="wp")
        dma(out=wp_sb[:], in_=d["wproj"].ap().rearrange("(pp p) n -> p pp n", p=128))

        # ---- phase 3: attention in S^T layout, head pairs, K=128 via padded kT ----
        with (
            tc.tile_pool(name="es", bufs=4) as espool,
            tc.tile_pool(name="rc", bufs=2) as rcpool,
            tc.tile_pool(name="pss", bufs=2, space="PSUM") as pss,
            tc.tile_pool(name="psy", bufs=2, space="PSUM") as psy,
        ):
            for p in range(4):
                rsums = [
                    rcpool.tile([1, T], F32, name=f"rsum{h2}", tag=f"rsum{h2}")
                    for h2 in (0, 1)
                ]
                pending = None  # deferred (yqs, qc) tail copies

                def flush_tail(pend):
                    yq_t, qc_t = pend
                    for h2 in (0, 1):
                        pr = 64 * h2
                        nc.vector.tensor_copy(
                            out=yT[p][pr : pr + 64, 512 * qc_t : 512 * qc_t + 512],
                            in_=yq_t[h2][0:64, :],
                        )
                        nc.vector.tensor_copy(
                            out=rsums[h2][:, 512 * qc_t : 512 * qc_t + 512],
                            in_=yq_t[h2][64:65, :],
                        )

                for qc in range(4):
                    yqs = [
                        psy.tile([65, 512], F32, name=f"yq{h2}", tag=f"yq{h2}")
                        for h2 in (0, 1)
                    ]
                    nki = 4 * qc + 4
                    for ki in range(nki):
                        sblk = pss.tile([128, 1024], F32, name="sblk", tag="sblk")
                        for h2 in (0, 1):
                            nc.tensor.matmul(
                                sblk[:, 512 * h2 : 512 * h2 + 512],
                                kTp[p][h2][:, 128 * ki : 128 * ki + 128],
                                qT[p][:, 512 * qc : 512 * qc + 512],
                                start=True,
                                stop=True,
                            )
                        r = ki - 4 * qc
                        es = espool.tile([128, 1024], BF16, name="es", tag="es")
                        if r >= 1:
                            # exp only the causally-reachable region of each half
                            w = 512 - 128 * r
                            sv = sblk[:, 128 * r : 128 * r + w]
                            s2 = bass.AP(
                                tensor=sv.tensor,
                                offset=sv.offset,
                                ap=[list(sv.ap[0]), [512, 2], list(sv.ap[1])],
                            )
                            ev = es[:, 128 * r : 128 * r + w]
                            e2 = bass.AP(
                                tensor=ev.tensor,
                                offset=ev.offset,
                                ap=[list(ev.ap[0]), [512, 2], list(ev.ap[1])],
                            )
                            nc.scalar.activation(e2, s2, Exp)
                        else:
                            nc.scalar.activation(es[:], sblk[:], Exp)
                        if r >= 0:
                            # zero the upper triangle of the 128-wide boundary block
                            for h2 in (0, 1):
                                c0 = 512 * h2 + 128 * r
                                nc.vector.tensor_tensor(
                                    es[:, c0 : c0 + 128],
                                    es[:, c0 : c0 + 128],
                                    m01_sb[:],
                                    mult,
                                )
                        first, last = ki == 0, ki == nki - 1
                        vs = max(0, 128 * r)
                        for h2 in (0, 1):
                            hl = 2 * p + h2
                            nc.tensor.matmul(
                                yqs[h2][:, vs:512],
                                Vt[ki][:, 65 * hl : 65 * hl + 65],
                                es[:, 512 * h2 + vs : 512 * h2 + 512],
                                start=first,
                                stop=last,
                                skip_group_check=True,
                            )
                        if ki == 0 and pending is not None:
                            flush_tail(pending)
                            pending = None
                    pending = (yqs, qc)
                flush_tail(pending)
                pending = None
                for h2 in (0, 1):
                    pr = 64 * h2
                    rs8 = rcpool.tile([64, 32], F32, name="rs8", tag="rs8")
                    dma(out=rs8[:], in_=rsums[h2][:])
                    rr8 = rcpool.tile([64, 32], F32, name="rr8", tag="rr8")
                    nc.vector.reciprocal(rr8[:], rs8[:])
                    dma(out=rsums[h2][:], in_=rr8[:])
                    rcb = rcpool.tile([128, T], F32, name="rcb", tag="rcb", bufs=1)
                    nc.gpsimd.partition_broadcast(rcb[:], rsums[h2][:])
                    nc.vector.tensor_tensor(
                        yT[p][pr : pr + 64, :],
                        yT[p][pr : pr + 64, :],
                        rcb[pr : pr + 64, :],
                        mult,
                    )

        # ---- phase 4: out-projection (row-sharded partial) ----
        with (
            tc.tile_pool(name="osb", bufs=3) as opool,
            tc.tile_pool(name="pso", bufs=4, space="PSUM") as pso,
        ):
            for tt in range(TT):
                for cc in range(2):
                    ps = pso.tile([128, 512], F32, name="pso", tag="pso")
                    for pp in range(4):
                        nc.tensor.matmul(
                            ps[:],
                            yT[pp][:, 128 * tt : 128 * tt + 128],
                            wp_sb[:, pp, 512 * cc : 512 * cc + 512],
                            start=(pp == 0),
                            stop=(pp == 3),
                        )
                    ob = opool.tile([128, 512], F32, name="ob", tag="ob")
                    nc.scalar.copy(ob[:], ps[:])
                    dma(
                        out=d["out"].ap()[
                            128 * tt : 128 * tt + 128, 512 * cc : 512 * cc + 512
                        ],
                        in_=ob[:],
                    )
        wppool.release()


def _build():
    nc = bacc.Bacc("TRN2", target_bir_lowering=False, debug=False, num_devices=N_CORES)
    d = {
        "xT": nc.dram_tensor("xT", [C, T], F32R, kind="ExternalInput"),
        "wq": nc.dram_tensor("wq", [C, 512], F32R, kind="ExternalInput"),
        "wk": nc.dram_tensor("wk", [C, 512], F32R, kind="ExternalInput"),
        "wv": nc.dram_tensor("wv", [C, 520], F32R, kind="ExternalInput"),
        "bv": nc.dram_tensor("bv", [128, 520], F32, kind="ExternalInput"),
        "bq": nc.dram_tensor("bq", [128, 4], F32, kind="ExternalInput"),
        "bk": nc.dram_tensor("bk", [128, 4], F32, kind="ExternalInput"),
        "m01": nc.dram_tensor("m01", [128, 128], mybir.dt.bfloat16, kind="ExternalInput"),
        "wproj": nc.dram_tensor("wproj", [512, 1024], F32R, kind="ExternalInput"),
        "out": nc.dram_tensor("out", [T, C], F32, kind="ExternalOutput"),
    }
    with tile.TileContext(nc) as tcx:
        _emit(nc, tcx, d)
    nc.compile()
    return nc


def _prep_core_inputs(c, x, w_attn, b_attn):
    g = c % 2
    xT = np.ascontiguousarray(x[c // 2].T)
    wq = np.ascontiguousarray(w_attn[:, 512 * g : 512 * g + 512])
    wk = np.ascontiguousarray(w_attn[:, 1024 + 512 * g : 1024 + 512 * g + 512])
    wv = np.zeros((C, 520), np.float32)
    bv = np.zeros((128, 520), np.float32)
    for hl in range(8):
        hcol = 2048 + 512 * g + 64 * hl
        wv[:, 65 * hl : 65 * hl + 64] = w_attn[:, hcol : hcol + 64]
        bv[:, 65 * hl : 65 * hl + 64] = b_attn[hcol : hcol + 64][None, :]
        bv[:, 65 * hl + 64] = 1.0
    bq = np.zeros((128, 4), np.float32)
    bk = np.zeros((128, 4), np.float32)
    for p in range(4):
        bq[:, p] = b_attn[512 * g + 128 * p : 512 * g + 128 * p + 128] * 0.125
        bk[:, p] = b_attn[1024 + 512 * g + 128 * p : 1024 + 512 * g + 128 * p + 128]
    import ml_dtypes

    m01 = (
        np.arange(128)[:, None] <= np.arange(128)[None, :]
    ).astype(ml_dtypes.bfloat16)
    return dict(xT=xT, wq=wq, wk=wk, wv=wv, bv=bv, bq=bq, bk=bk, m01=m01)


def make_in_maps(x, w_attn, b_attn, w_proj):
    x = np.asarray(x, np.float32)
    w_attn = np.asarray(w_attn, np.float32)
    b_attn = np.asarray(b_attn, np.float32)
    w_proj = np.asarray(w_proj, np.float32)
    in_maps = []
    for c in range(N_CORES):
        m = _prep_core_inputs(c, x, w_attn, b_attn)
        g = c % 2
        m["wproj"] = np.ascontiguousarray(w_proj[512 * g : 512 * g + 512, :])
        in_maps.append(m)
    return in_maps


def get_nc():
    if "nc" not in _cache:
        _cache["nc"] = _build()
    return _cache["nc"]


def gather(results, b_proj):
    b_proj = np.asarray(b_proj, np.float32)
    full = np.empty((B, T, C), np.float32)
    for b in range(B):
        full[b] = results[2 * b]["out"] + results[2 * b + 1]["out"] + b_proj[None, :]
    return full


def kernel(x, w_attn, b_attn, w_proj, b_proj):
    nc = get_nc()
    in_maps = make_in_maps(x, w_attn, b_attn, w_proj)
    res = run_bass_kernel_spmd(nc, in_maps, list(range(N_CORES)))
    return gather(res.results, b_proj)


# revision 18
# speedup vs baseline: 3.0977x; 1.0139x over previous
"""Causal self-attention (B=4, T=2048, C=1024, 16 heads) on 8 trn2 NeuronCores.

Sharding: core c = (batch c//2, head-group c%2 of 8 heads). Data-parallel over
batch, tensor-parallel over heads; out-proj is row-sharded and the two partial
products per batch are summed on the host (no device collectives).

Device program per core (all fp32):
  phase 1: q^T/k^T = W^T @ x^T as head-pair tiles [128, T] (d on partitions)
  phase 2: V in natural [t, d] layout, augmented with a ones column per head
           (bias trick) so P@V also accumulates softmax row-sums for free
  phase 3: flash-style attention in S^T layout (S computed transposed — no PE
           transposes, no max subtraction: |S| < ~3 by construction), causal
           masking via a single static 128x128 triangular mask + memsets,
           normalization folded into the PSUM->SBUF copy
  phase 4: out = y^T-slices^T @ W_proj rows (partial over this core's heads)
"""

import os
import sys

import numpy as np

for _p in ("/opt/trn_rl_repo", "/root/.axon_site/_ro/trn_rl_repo"):
    if os.path.isdir(_p) and _p not in sys.path:
        sys.path.insert(0, _p)

import concourse.bass as bass  # noqa: E402
import concourse.tile as tile  # noqa: E402
from concourse import bacc, mybir  # noqa: E402
from concourse.bass_utils import run_bass_kernel_spmd  # noqa: E402

B, T, C = 4, 2048, 1024
H, D = 16, 64
N_CORES = 8
F32 = mybir.dt.float32
F32R = mybir.dt.float32r
BF16 = mybir.dt.bfloat16


def _r(ap):
    return ap.bitcast(F32R)
TC = T // 512  # 4 t-chunks of 512
TT = T // 128  # 16 t-tiles of 128
CT = C // 128  # 8 c-tiles of 128

_cache: dict = {}


def _emit(nc: "bacc.Bacc", tc: "tile.TileContext", d: dict) -> None:
    mult = mybir.AluOpType.mult
    add = mybir.AluOpType.add
    Exp = mybir.ActivationFunctionType.Exp
    dma = nc.sync.dma_start

    with (
        tc.tile_pool(name="const", bufs=1) as cpool,
        tc.tile_pool(name="persist", bufs=1) as persist,
    ):
        m01_sb = cpool.tile([128, 128], BF16, name="m01", tag="m01")
        dma(out=m01_sb[:], in_=d["m01"].ap())
        warm_sb = cpool.tile([128, 2], F32, name="warm", tag="warm")
        nc.vector.memset(warm_sb[:, 0:1], 0.0)
        nc.scalar.activation(
            warm_sb[:, 1:2], warm_sb[:, 0:1], mybir.ActivationFunctionType.Exp
        )
        bq_sb = cpool.tile([128, 4], F32, name="bq", tag="bq")
        dma(out=bq_sb[:], in_=d["bq"].ap())
        bk_sb = cpool.tile([128, 4], F32, name="bk", tag="bk")
        dma(out=bk_sb[:], in_=d["bk"].ap())
        bv_sb = cpool.tile([128, 520], F32, name="bv", tag="bv")
        dma(out=bv_sb[:], in_=d["bv"].ap())

        qT = [persist.tile([128, T], BF16, name=f"qT{p}", tag=f"qT{p}") for p in range(4)]
        kTp = [
            [
                persist.tile([128, T], BF16, name=f"kT{p}_{h2}", tag=f"kT{p}_{h2}")
                for h2 in (0, 1)
            ]
            for p in range(4)
        ]
        Vt = [persist.tile([128, 520], BF16, name=f"V{i}", tag=f"V{i}") for i in range(TT)]
        yT = [persist.tile([128, T], F32R, name=f"yT{p}", tag=f"yT{p}") for p in range(4)]
        for p in range(4):
            nc.gpsimd.memset(kTp[p][0][64:128, :], 0.0)
            nc.gpsimd.memset(kTp[p][1][0:64, :], 0.0)

        # ---- phase 1: q^T / k^T head-pair tiles ----
        with (
            tc.tile_pool(name="wqk", bufs=1) as wpool,
            tc.tile_pool(name="xt1", bufs=2) as xpool,
            tc.tile_pool(name="psqk", bufs=4, space="PSUM") as pq,
        ):
            wq_sb = wpool.tile([128, CT, 512], F32R, name="wq", tag="wq")
            dma(out=wq_sb[:], in_=d["wq"].ap().rearrange("(ct p) n -> p ct n", p=128))
            wk_sb = wpool.tile([128, CT, 512], F32R, name="wk", tag="wk")
            dma(out=wk_sb[:], in_=d["wk"].ap().rearrange("(ct p) n -> p ct n", p=128))
            for tci in range(TC):
                xt = xpool.tile([128, CT, 512], F32R, name="xt", tag="xt")
                dma(
                    out=xt[:],
                    in_=d["xT"].ap()[:, 512 * tci : 512 * tci + 512].rearrange(
                        "(ct p) n -> p ct n", p=128
                    ),
                )
                for p in range(4):
                    for iw, w_sb in ((0, wq_sb), (1, wk_sb)):
                        ps = pq.tile([128, 512], F32, name="psqk", tag="psqk")
                        for ct in range(CT):
                            nc.tensor.matmul(
                                ps[:],
                                w_sb[:, ct, 128 * p : 128 * p + 128],
                                xt[:, ct, :],
                                start=(ct == 0),
                                stop=(ct == CT - 1),
                            )
                        if iw == 0:
                            nc.vector.tensor_scalar(
                                qT[p][:, 512 * tci : 512 * tci + 512],
                                ps[:],
                                0.125,
                                bq_sb[:, p : p + 1],
                                mult,
                                add,
                            )
                        else:
                            for h2 in (0, 1):
                                pr = 64 * h2
                                nc.vector.tensor_scalar(
                                    kTp[p][h2][
                                        pr : pr + 64, 512 * tci : 512 * tci + 512
                                    ],
                                    ps[pr : pr + 64, :],
                                    1.0,
                                    bk_sb[pr : pr + 64, p : p + 1],
                                    mult,
                                    add,
                                )

        # ---- phase 2: V (natural layout, ones-augmented) ----
        with (
            tc.tile_pool(name="wv", bufs=1) as wvpool,
            tc.tile_pool(name="xt2", bufs=2) as xpool2,
            tc.tile_pool(name="psv", bufs=4, space="PSUM") as pv,
        ):
            wv_sb = wvpool.tile([128, CT, 520], F32R, name="wv", tag="wv")
            dma(out=wv_sb[:], in_=d["wv"].ap().rearrange("(ct p) n -> p ct n", p=128))
            for tci in range(TC):
                xt = xpool2.tile([128, CT, 512], F32R, name="xt2", tag="xt2")
                dma(
                    out=xt[:],
                    in_=d["xT"].ap()[:, 512 * tci : 512 * tci + 512].rearrange(
                        "(ct p) n -> p ct n", p=128
                    ),
                )
                for tt in range(4):
                    for qd in range(2):
                        ps = pv.tile([128, 260], F32, name="psv", tag="psv")
                        for ct in range(CT):
                            nc.tensor.matmul(
                                ps[:],
                                xt[:, ct, 128 * tt : 128 * tt + 128],
                                wv_sb[:, ct, 260 * qd : 260 * qd + 260],
                                start=(ct == 0),
                                stop=(ct == CT - 1),
                            )
                        nc.vector.tensor_tensor(
                            Vt[4 * tci + tt][:, 260 * qd : 260 * qd + 260],
                            ps[:],
                            bv_sb[:, 260 * qd : 260 * qd + 260],
                            add,
                        )

        wppool = tc.alloc_tile_pool(name="wpj", bufs=1)
        wp_sb = wppool.tile([128, 4, 1024], F32R, name="wp", tag="wp")
        dma(out=wp_sb[:], in_=d["wproj"].ap().rearrange("(pp p) n -> p pp n", p=128))

        # ---- phase 3: attention in S^T layout, head pairs, K=128 via padded kT ----
        with (
            tc.tile_pool(name="es", bufs=6) as espool,
            tc.tile_pool(name="rc", bufs=4) as rcpool,
            tc.tile_pool(name="pss", bufs=2, space="PSUM") as pss,
            tc.tile_pool(name="psy", bufs=2, space="PSUM") as psy,
        ):
            for p in range(4):
                pending = None  # deferred (yqs, qc) tail copies

                def flush_tail(pend, p=None):
                    yq_t, qc_t = pend
                    for h2 in (0, 1):
                        pr = 64 * h2
                        cs = 512 * qc_t
                        nc.vector.tensor_copy(
                            out=yT[p][pr : pr + 64, cs : cs + 512],
                            in_=yq_t[h2][0:64, :],
                        )
                        rsum = rcpool.tile([1, 512], F32, name="rsum", tag="rsum")
                        nc.vector.tensor_copy(out=rsum[:], in_=yq_t[h2][64:65, :])
                        rs8 = rcpool.tile([64, 8], F32, name="rs8", tag="rs8")
                        dma(out=rs8[:], in_=rsum[:])
                        rr8 = rcpool.tile([64, 8], F32, name="rr8", tag="rr8")
                        nc.vector.reciprocal(rr8[:], rs8[:])
                        dma(out=rsum[:], in_=rr8[:])
                        rcb = rcpool.tile([128, 512], F32, name="rcb", tag="rcb")
                        nc.gpsimd.partition_broadcast(rcb[:], rsum[:])
                        nc.vector.tensor_tensor(
                            yT[p][pr : pr + 64, cs : cs + 512],
                            yT[p][pr : pr + 64, cs : cs + 512],
                            rcb[pr : pr + 64, :],
                            mult,
                        )

                for qc in range(4):
                    yqs = [
                        psy.tile([65, 512], F32, name=f"yq{h2}", tag=f"yq{h2}")
                        for h2 in (0, 1)
                    ]
                    nki = 4 * qc + 4
                    for ki in range(nki):
                        sblk = pss.tile([128, 1024], F32, name="sblk", tag="sblk")
                        for h2 in (0, 1):
                            nc.tensor.matmul(
                                sblk[:, 512 * h2 : 512 * h2 + 512],
                                kTp[p][h2][:, 128 * ki : 128 * ki + 128],
                                qT[p][:, 512 * qc : 512 * qc + 512],
                                start=True,
                                stop=True,
                            )
                        r = ki - 4 * qc
                        es = espool.tile([128, 1024], BF16, name="es", tag="es")
                        if r >= 1:
                            # exp only the causally-reachable region of each half
                            w = 512 - 128 * r
                            sv = sblk[:, 128 * r : 128 * r + w]
                            s2 = bass.AP(
                                tensor=sv.tensor,
                                offset=sv.offset,
                                ap=[list(sv.ap[0]), [512, 2], list(sv.ap[1])],
                            )
                            ev = es[:, 128 * r : 128 * r + w]
                            e2 = bass.AP(
                                tensor=ev.tensor,
                                offset=ev.offset,
                                ap=[list(ev.ap[0]), [512, 2], list(ev.ap[1])],
                            )
                            nc.scalar.activation(e2, s2, Exp)
                        else:
                            nc.scalar.activation(es[:], sblk[:], Exp)
                        if r >= 0:
                            # zero the upper triangle of the 128-wide boundary block
                            for h2 in (0, 1):
                                c0 = 512 * h2 + 128 * r
                                nc.vector.tensor_tensor(
                                    es[:, c0 : c0 + 128],
                                    es[:, c0 : c0 + 128],
                                    m01_sb[:],
                                    mult,
                                )
                        first, last = ki == 0, ki == nki - 1
                        vs = max(0, 128 * r)
                        for h2 in (0, 1):
                            hl = 2 * p + h2
                            nc.tensor.matmul(
                                yqs[h2][:, vs:512],
                                Vt[ki][:, 65 * hl : 65 * hl + 65],
                                es[:, 512 * h2 + vs : 512 * h2 + 512],
                                start=first,
                                stop=last,
                                skip_group_check=True,
                            )
                        if ki == 0 and pending is not None:
                            flush_tail(pending, p)
                            pending = None
                    pending = (yqs, qc)
                flush_tail(pending, p)
                pending = None

        # ---- phase 4: out-projection (row-sharded partial) ----
        with (
            tc.tile_pool(name="osb", bufs=3) as opool,
            tc.tile_pool(name="pso", bufs=4, space="PSUM") as pso,
        ):
            for tt in range(TT):
                for cc in range(2):
                    ps = pso.tile([128, 512], F32, name="pso", tag="pso")
                    for pp in range(4):
                        nc.tensor.matmul(
                            ps[:],
                            yT[pp][:, 128 * tt : 128 * tt + 128],
                            wp_sb[:, pp, 512 * cc : 512 * cc + 512],
                            start=(pp == 0),
                            stop=(pp == 3),
                        )
                    ob = opool.tile([128, 512], F32, name="ob", tag="ob")
                    nc.scalar.copy(ob[:], ps[:])
                    dma(
                        out=d["out"].ap()[
                            128 * tt : 128 * tt + 128, 512 * cc : 512 * cc + 512
                        ],
                        in_=ob[:],
                    )
        wppool.release()


def _build():
    nc = bacc.Bacc("TRN2", target_bir_lowering=False, debug=False, num_devices=N_CORES)
    d = {
        "xT": nc.dram_tensor("xT", [C, T], F32R, kind="ExternalInput"),
        "wq": nc.dram_tensor("wq", [C, 512], F32R, kind="ExternalInput"),
        "wk": nc.dram_tensor("wk", [C, 512], F32R, kind="ExternalInput"),
        "wv": nc.dram_tensor("wv", [C, 520], F32R, kind="ExternalInput"),
        "bv": nc.dram_tensor("bv", [128, 520], F32, kind="ExternalInput"),
        "bq": nc.dram_tensor("bq", [128, 4], F32, kind="ExternalInput"),
        "bk": nc.dram_tensor("bk", [128, 4], F32, kind="ExternalInput"),
        "m01": nc.dram_tensor("m01", [128, 128], mybir.dt.bfloat16, kind="ExternalInput"),
        "wproj": nc.dram_tensor("wproj", [512, 1024], F32R, kind="ExternalInput"),
        "out": nc.dram_tensor("out", [T, C], F32, kind="ExternalOutput"),
    }
    with tile.TileContext(nc) as tcx:
        _emit(nc, tcx, d)
    nc.compile()
    return nc


def _prep_core_inputs(c, x, w_attn, b_attn):
    g = c % 2
    xT = np.ascontiguousarray(x[c // 2].T)
    wq = np.ascontiguousarray(w_attn[:, 512 * g : 512 * g + 512])
    wk = np.ascontiguousarray(w_attn[:, 1024 + 512 * g : 1024 + 512 * g + 512])
    wv = np.zeros((C, 520), np.float32)
    bv = np.zeros((128, 520), np.float32)
    for hl in range(8):
        hcol = 2048 + 512 * g + 64 * hl
        wv[:, 65 * hl : 65 * hl + 64] = w_attn[:, hcol : hcol + 64]
        bv[:, 65 * hl : 65 * hl + 64] = b_attn[hcol : hcol + 64][None, :]
        bv[:, 65 * hl + 64] = 1.0
    bq = np.zeros((128, 4), np.float32)
    bk = np.zeros((128, 4), np.float32)
    for p in range(4):
        bq[:, p] = b_attn[512 * g + 128 * p : 512 * g + 128 * p + 128] * 0.125
        bk[:, p] = b_attn[1024 + 512 * g + 128 * p : 1024 + 512 * g + 128 * p + 128]
    import ml_dtypes

    m01 = (
        np.arange(128)[:, None] <= np.arange(128)[None, :]
    ).astype(ml_dtypes.bfloat16)
    return dict(xT=xT, wq=wq, wk=wk, wv=wv, bv=bv, bq=bq, bk=bk, m01=m01)


def make_in_maps(x, w_attn, b_attn, w_proj):
    x = np.asarray(x, np.float32)
    w_attn = np.asarray(w_attn, np.float32)
    b_attn = np.asarray(b_attn, np.float32)
    w_proj = np.asarray(w_proj, np.float32)
    in_maps = []
    for c in range(N_CORES):
        m = _prep_core_inputs(c, x, w_attn, b_attn)
        g = c % 2
        m["wproj"] = np.ascontiguousarray(w_proj[512 * g : 512 * g + 512, :])
        in_maps.append(m)
    return in_maps


def get_nc():
    if "nc" not in _cache:
        _cache["nc"] = _build()
    return _cache["nc"]


def gather(results, b_proj):
    b_proj = np.asarray(b_proj, np.float32)
    full = np.empty((B, T, C), np.float32)
    for b in range(B):
        full[b] = results[2 * b]["out"] + results[2 * b + 1]["out"] + b_proj[None, :]
    return full


def kernel(x, w_attn, b_attn, w_proj, b_proj):
    nc = get_nc()
    in_maps = make_in_maps(x, w_attn, b_attn, w_proj)
    res = run_bass_kernel_spmd(nc, in_maps, list(range(N_CORES)))
    return gather(res.results, b_proj)


# revision 19
# speedup vs baseline: 3.2618x; 1.0530x over previous
"""Causal self-attention (B=4, T=2048, C=1024, 16 heads) on 8 trn2 NeuronCores.

Sharding: core c = (batch c//2, head-group c%2 of 8 heads). Data-parallel over
batch, tensor-parallel over heads; out-proj is row-sharded and the two partial
products per batch are summed on the host (no device collectives).

Device program per core (all fp32):
  phase 1: q^T/k^T = W^T @ x^T as head-pair tiles [128, T] (d on partitions)
  phase 2: V in natural [t, d] layout, augmented with a ones column per head
           (bias trick) so P@V also accumulates softmax row-sums for free
  phase 3: flash-style attention in S^T layout (S computed transposed — no PE
           transposes, no max subtraction: |S| < ~3 by construction), causal
           masking via a single static 128x128 triangular mask + memsets,
           normalization folded into the PSUM->SBUF copy
  phase 4: out = y^T-slices^T @ W_proj rows (partial over this core's heads)
"""

import os
import sys

import numpy as np

for _p in ("/opt/trn_rl_repo", "/root/.axon_site/_ro/trn_rl_repo"):
    if os.path.isdir(_p) and _p not in sys.path:
        sys.path.insert(0, _p)

import concourse.bass as bass  # noqa: E402
import concourse.tile as tile  # noqa: E402
from concourse import bacc, mybir  # noqa: E402
from concourse.bass_utils import run_bass_kernel_spmd  # noqa: E402

B, T, C = 4, 2048, 1024
H, D = 16, 64
N_CORES = 8
F32 = mybir.dt.float32
F32R = mybir.dt.float32r
BF16 = mybir.dt.bfloat16


def _r(ap):
    return ap.bitcast(F32R)
TC = T // 512  # 4 t-chunks of 512
TT = T // 128  # 16 t-tiles of 128
CT = C // 128  # 8 c-tiles of 128

_cache: dict = {}


def _emit(nc: "bacc.Bacc", tc: "tile.TileContext", d: dict) -> None:
    mult = mybir.AluOpType.mult
    add = mybir.AluOpType.add
    Exp = mybir.ActivationFunctionType.Exp
    dma = nc.sync.dma_start

    with (
        tc.tile_pool(name="const", bufs=1) as cpool,
        tc.tile_pool(name="persist", bufs=1) as persist,
    ):
        m01_sb = cpool.tile([128, 128], BF16, name="m01", tag="m01")
        dma(out=m01_sb[:], in_=d["m01"].ap())
        warm_sb = cpool.tile([128, 2], F32, name="warm", tag="warm")
        nc.vector.memset(warm_sb[:, 0:1], 0.0)
        nc.scalar.activation(
            warm_sb[:, 1:2], warm_sb[:, 0:1], mybir.ActivationFunctionType.Exp
        )
        bq_sb = cpool.tile([128, 4], F32, name="bq", tag="bq")
        dma(out=bq_sb[:], in_=d["bq"].ap())
        bk_sb = cpool.tile([128, 4], F32, name="bk", tag="bk")
        dma(out=bk_sb[:], in_=d["bk"].ap())
        bv_sb = cpool.tile([128, 520], F32, name="bv", tag="bv")
        dma(out=bv_sb[:], in_=d["bv"].ap())

        qT = [persist.tile([128, T], BF16, name=f"qT{p}", tag=f"qT{p}") for p in range(4)]
        kTp = [
            [
                persist.tile([128, T], BF16, name=f"kT{p}_{h2}", tag=f"kT{p}_{h2}")
                for h2 in (0, 1)
            ]
            for p in range(4)
        ]
        Vt = [persist.tile([128, 520], BF16, name=f"V{i}", tag=f"V{i}") for i in range(TT)]
        for p in range(4):
            nc.gpsimd.memset(kTp[p][0][64:128, :], 0.0)
            nc.gpsimd.memset(kTp[p][1][0:64, :], 0.0)

        # ---- phase 1+2: q^T/k^T head-pair tiles + V (shared x loads) ----
        with (
            tc.tile_pool(name="wqkv", bufs=1) as wpool,
            tc.tile_pool(name="xt1", bufs=2) as xpool,
            tc.tile_pool(name="psqk", bufs=3, space="PSUM") as pq,
            tc.tile_pool(name="psv", bufs=3, space="PSUM") as pv,
        ):
            wq_sb = wpool.tile([128, CT, 512], F32R, name="wq", tag="wq")
            dma(out=wq_sb[:], in_=d["wq"].ap().rearrange("(ct p) n -> p ct n", p=128))
            wk_sb = wpool.tile([128, CT, 512], F32R, name="wk", tag="wk")
            dma(out=wk_sb[:], in_=d["wk"].ap().rearrange("(ct p) n -> p ct n", p=128))
            wv_sb = wpool.tile([128, CT, 520], F32R, name="wv", tag="wv")
            dma(out=wv_sb[:], in_=d["wv"].ap().rearrange("(ct p) n -> p ct n", p=128))
            for tci in range(TC):
                xt = xpool.tile([128, CT, 512], F32R, name="xt", tag="xt")
                dma(
                    out=xt[:],
                    in_=d["xT"].ap()[:, 512 * tci : 512 * tci + 512].rearrange(
                        "(ct p) n -> p ct n", p=128
                    ),
                )
                for p in range(4):
                    for iw, w_sb in ((0, wq_sb), (1, wk_sb)):
                        ps = pq.tile([128, 512], F32, name="psqk", tag="psqk")
                        for ct in range(CT):
                            nc.tensor.matmul(
                                ps[:],
                                w_sb[:, ct, 128 * p : 128 * p + 128],
                                xt[:, ct, :],
                                start=(ct == 0),
                                stop=(ct == CT - 1),
                            )
                        if iw == 0:
                            nc.vector.tensor_scalar(
                                qT[p][:, 512 * tci : 512 * tci + 512],
                                ps[:],
                                0.125,
                                bq_sb[:, p : p + 1],
                                mult,
                                add,
                            )
                        else:
                            for h2 in (0, 1):
                                pr = 64 * h2
                                nc.vector.tensor_scalar(
                                    kTp[p][h2][
                                        pr : pr + 64, 512 * tci : 512 * tci + 512
                                    ],
                                    ps[pr : pr + 64, :],
                                    1.0,
                                    bk_sb[pr : pr + 64, p : p + 1],
                                    mult,
                                    add,
                                )
                for tt in range(4):
                    for qd in range(2):
                        ps = pv.tile([128, 260], F32, name="psv", tag="psv")
                        for ct in range(CT):
                            nc.tensor.matmul(
                                ps[:],
                                xt[:, ct, 128 * tt : 128 * tt + 128],
                                wv_sb[:, ct, 260 * qd : 260 * qd + 260],
                                start=(ct == 0),
                                stop=(ct == CT - 1),
                            )
                        nc.vector.tensor_tensor(
                            Vt[4 * tci + tt][:, 260 * qd : 260 * qd + 260],
                            ps[:],
                            bv_sb[:, 260 * qd : 260 * qd + 260],
                            add,
                        )

        wppool = tc.alloc_tile_pool(name="wpj", bufs=1)
        wp_sb = wppool.tile([128, 4, 1024], F32R, name="wp", tag="wp")
        dma(out=wp_sb[:], in_=d["wproj"].ap().rearrange("(pp p) n -> p pp n", p=128))
        ypool = tc.alloc_tile_pool(name="ypool", bufs=1)
        yT = [ypool.tile([128, T], F32R, name=f"yT{p}", tag=f"yT{p}") for p in range(4)]

        # ---- phase 3: attention in S^T layout, head pairs, K=128 via padded kT ----
        with (
            tc.tile_pool(name="es", bufs=6) as espool,
            tc.tile_pool(name="rc", bufs=4) as rcpool,
            tc.tile_pool(name="pss", bufs=2, space="PSUM") as pss,
            tc.tile_pool(name="psy", bufs=2, space="PSUM") as psy,
        ):
            for p in range(4):
                pending = None  # deferred (yqs, qc) tail copies

                def flush_tail(pend, p=None):
                    yq_t, qc_t = pend
                    for h2 in (0, 1):
                        pr = 64 * h2
                        cs = 512 * qc_t
                        nc.vector.tensor_copy(
                            out=yT[p][pr : pr + 64, cs : cs + 512],
                            in_=yq_t[h2][0:64, :],
                        )
                        rsum = rcpool.tile([1, 512], F32, name="rsum", tag="rsum")
                        nc.vector.tensor_copy(out=rsum[:], in_=yq_t[h2][64:65, :])
                        rs8 = rcpool.tile([64, 8], F32, name="rs8", tag="rs8")
                        dma(out=rs8[:], in_=rsum[:])
                        rr8 = rcpool.tile([64, 8], F32, name="rr8", tag="rr8")
                        nc.vector.reciprocal(rr8[:], rs8[:])
                        dma(out=rsum[:], in_=rr8[:])
                        rcb = rcpool.tile([128, 512], F32, name="rcb", tag="rcb")
                        nc.gpsimd.partition_broadcast(rcb[:], rsum[:])
                        nc.vector.tensor_tensor(
                            yT[p][pr : pr + 64, cs : cs + 512],
                            yT[p][pr : pr + 64, cs : cs + 512],
                            rcb[pr : pr + 64, :],
                            mult,
                        )

                for qc in range(4):
                    yqs = [
                        psy.tile([65, 512], F32, name=f"yq{h2}", tag=f"yq{h2}")
                        for h2 in (0, 1)
                    ]
                    nki = 4 * qc + 4
                    for ki in range(nki):
                        sblk = pss.tile([128, 1024], F32, name="sblk", tag="sblk")
                        for h2 in (0, 1):
                            nc.tensor.matmul(
                                sblk[:, 512 * h2 : 512 * h2 + 512],
                                kTp[p][h2][:, 128 * ki : 128 * ki + 128],
                                qT[p][:, 512 * qc : 512 * qc + 512],
                                start=True,
                                stop=True,
                            )
                        r = ki - 4 * qc
                        es = espool.tile([128, 1024], BF16, name="es", tag="es")
                        if r >= 1:
                            # exp only the causally-reachable region of each half
                            w = 512 - 128 * r
                            sv = sblk[:, 128 * r : 128 * r + w]
                            s2 = bass.AP(
                                tensor=sv.tensor,
                                offset=sv.offset,
                                ap=[list(sv.ap[0]), [512, 2], list(sv.ap[1])],
                            )
                            ev = es[:, 128 * r : 128 * r + w]
                            e2 = bass.AP(
                                tensor=ev.tensor,
                                offset=ev.offset,
                                ap=[list(ev.ap[0]), [512, 2], list(ev.ap[1])],
                            )
                            nc.scalar.activation(e2, s2, Exp)
                        else:
                            nc.scalar.activation(es[:], sblk[:], Exp)
                        if r >= 0:
                            # zero the upper triangle of the 128-wide boundary block
                            for h2 in (0, 1):
                                c0 = 512 * h2 + 128 * r
                                nc.vector.tensor_tensor(
                                    es[:, c0 : c0 + 128],
                                    es[:, c0 : c0 + 128],
                                    m01_sb[:],
                                    mult,
                                )
                        first, last = ki == 0, ki == nki - 1
                        vs = max(0, 128 * r)
                        for h2 in (0, 1):
                            hl = 2 * p + h2
                            nc.tensor.matmul(
                                yqs[h2][:, vs:512],
                                Vt[ki][:, 65 * hl : 65 * hl + 65],
                                es[:, 512 * h2 + vs : 512 * h2 + 512],
                                start=first,
                                stop=last,
                                skip_group_check=True,
                            )
                        if ki == 0 and pending is not None:
                            flush_tail(pending, p)
                            pending = None
                    pending = (yqs, qc)
                flush_tail(pending, p)
                pending = None

        # ---- phase 4: out-projection (row-sharded partial) ----
        with (
            tc.tile_pool(name="osb", bufs=3) as opool,
            tc.tile_pool(name="pso", bufs=4, space="PSUM") as pso,
        ):
            for tt in range(TT):
                for cc in range(2):
                    ps = pso.tile([128, 512], F32, name="pso", tag="pso")
                    for pp in range(4):
                        nc.tensor.matmul(
                            ps[:],
                            yT[pp][:, 128 * tt : 128 * tt + 128],
                            wp_sb[:, pp, 512 * cc : 512 * cc + 512],
                            start=(pp == 0),
                            stop=(pp == 3),
                        )
                    ob = opool.tile([128, 512], F32, name="ob", tag="ob")
                    nc.scalar.copy(ob[:], ps[:])
                    dma(
                        out=d["out"].ap()[
                            128 * tt : 128 * tt + 128, 512 * cc : 512 * cc + 512
                        ],
                        in_=ob[:],
                    )
        ypool.release()
        wppool.release()


def _build():
    nc = bacc.Bacc("TRN2", target_bir_lowering=False, debug=False, num_devices=N_CORES)
    d = {
        "xT": nc.dram_tensor("xT", [C, T], F32R, kind="ExternalInput"),
        "wq": nc.dram_tensor("wq", [C, 512], F32R, kind="ExternalInput"),
        "wk": nc.dram_tensor("wk", [C, 512], F32R, kind="ExternalInput"),
        "wv": nc.dram_tensor("wv", [C, 520], F32R, kind="ExternalInput"),
        "bv": nc.dram_tensor("bv", [128, 520], F32, kind="ExternalInput"),
        "bq": nc.dram_tensor("bq", [128, 4], F32, kind="ExternalInput"),
        "bk": nc.dram_tensor("bk", [128, 4], F32, kind="ExternalInput"),
        "m01": nc.dram_tensor("m01", [128, 128], mybir.dt.bfloat16, kind="ExternalInput"),
        "wproj": nc.dram_tensor("wproj", [512, 1024], F32R, kind="ExternalInput"),
        "out": nc.dram_tensor("out", [T, C], F32, kind="ExternalOutput"),
    }
    with tile.TileContext(nc) as tcx:
        _emit(nc, tcx, d)
    nc.compile()
    return nc


def _prep_core_inputs(c, x, w_attn, b_attn):
    g = c % 2
    xT = np.ascontiguousarray(x[c // 2].T)
    wq = np.ascontiguousarray(w_attn[:, 512 * g : 512 * g + 512])
    wk = np.ascontiguousarray(w_attn[:, 1024 + 512 * g : 1024 + 512 * g + 512])
    wv = np.zeros((C, 520), np.float32)
    bv = np.zeros((128, 520), np.float32)
    for hl in range(8):
        hcol = 2048 + 512 * g + 64 * hl
        wv[:, 65 * hl : 65 * hl + 64] = w_attn[:, hcol : hcol + 64]
        bv[:, 65 * hl : 65 * hl + 64] = b_attn[hcol : hcol + 64][None, :]
        bv[:, 65 * hl + 64] = 1.0
    bq = np.zeros((128, 4), np.float32)
    bk = np.zeros((128, 4), np.float32)
    for p in range(4):
        bq[:, p] = b_attn[512 * g + 128 * p : 512 * g + 128 * p + 128] * 0.125
        bk[:, p] = b_attn[1024 + 512 * g + 128 * p : 1024 + 512 * g + 128 * p + 128]
    import ml_dtypes

    m01 = (
        np.arange(128)[:, None] <= np.arange(128)[None, :]
    ).astype(ml_dtypes.bfloat16)
    return dict(xT=xT, wq=wq, wk=wk, wv=wv, bv=bv, bq=bq, bk=bk, m01=m01)


def make_in_maps(x, w_attn, b_attn, w_proj):
    x = np.asarray(x, np.float32)
    w_attn = np.asarray(w_attn, np.float32)
    b_attn = np.asarray(b_attn, np.float32)
    w_proj = np.asarray(w_proj, np.float32)
    in_maps = []
    for c in range(N_CORES):
        m = _prep_core_inputs(c, x, w_attn, b_attn)
        g = c % 2
        m["wproj"] = np.ascontiguousarray(w_proj[512 * g : 512 * g + 512, :])
        in_maps.append(m)
    return in_maps


def get_nc():
    if "nc" not in _cache:
        _cache["nc"] = _build()
    return _cache["nc"]


def gather(results, b_proj):
    b_proj = np.asarray(b_proj, np.float32)
    full = np.empty((B, T, C), np.float32)
    for b in range(B):
        full[b] = results[2 * b]["out"] + results[2 * b + 1]["out"] + b_proj[None, :]
    return full


def kernel(x, w_attn, b_attn, w_proj, b_proj):
    nc = get_nc()
    in_maps = make_in_maps(x, w_attn, b_attn, w_proj)
    res = run_bass_kernel_spmd(nc, in_maps, list(range(N_CORES)))
    return gather(res.results, b_proj)


# revision 20
# speedup vs baseline: 3.3290x; 1.0206x over previous
"""Causal self-attention (B=4, T=2048, C=1024, 16 heads) on 8 trn2 NeuronCores.

Sharding: core c = (batch c//2, head-group c%2 of 8 heads). Data-parallel over
batch, tensor-parallel over heads; out-proj is row-sharded and the two partial
products per batch are summed on the host (no device collectives).

Device program per core (all fp32):
  phase 1: q^T/k^T = W^T @ x^T as head-pair tiles [128, T] (d on partitions)
  phase 2: V in natural [t, d] layout, augmented with a ones column per head
           (bias trick) so P@V also accumulates softmax row-sums for free
  phase 3: flash-style attention in S^T layout (S computed transposed — no PE
           transposes, no max subtraction: |S| < ~3 by construction), causal
           masking via a single static 128x128 triangular mask + memsets,
           normalization folded into the PSUM->SBUF copy
  phase 4: out = y^T-slices^T @ W_proj rows (partial over this core's heads)
"""

import os
import sys

import numpy as np

for _p in ("/opt/trn_rl_repo", "/root/.axon_site/_ro/trn_rl_repo"):
    if os.path.isdir(_p) and _p not in sys.path:
        sys.path.insert(0, _p)

import concourse.bass as bass  # noqa: E402
import concourse.tile as tile  # noqa: E402
from concourse import bacc, mybir  # noqa: E402
from concourse.bass_utils import run_bass_kernel_spmd  # noqa: E402

B, T, C = 4, 2048, 1024
H, D = 16, 64
N_CORES = 8
F32 = mybir.dt.float32
F32R = mybir.dt.float32r
BF16 = mybir.dt.bfloat16


def _r(ap):
    return ap.bitcast(F32R)
TC = T // 512  # 4 t-chunks of 512
TT = T // 128  # 16 t-tiles of 128
CT = C // 128  # 8 c-tiles of 128

_cache: dict = {}


def _emit(nc: "bacc.Bacc", tc: "tile.TileContext", d: dict) -> None:
    mult = mybir.AluOpType.mult
    add = mybir.AluOpType.add
    Exp = mybir.ActivationFunctionType.Exp
    dma = nc.sync.dma_start

    with (
        tc.tile_pool(name="const", bufs=1) as cpool,
        tc.tile_pool(name="persist", bufs=1) as persist,
    ):
        m01_sb = cpool.tile([128, 128], BF16, name="m01", tag="m01")
        dma(out=m01_sb[:], in_=d["m01"].ap())
        warm_sb = cpool.tile([128, 2], F32, name="warm", tag="warm")
        nc.vector.memset(warm_sb[:, 0:1], 0.0)
        nc.scalar.activation(
            warm_sb[:, 1:2], warm_sb[:, 0:1], mybir.ActivationFunctionType.Exp
        )
        bq_sb = cpool.tile([128, 4], F32, name="bq", tag="bq")
        dma(out=bq_sb[:], in_=d["bq"].ap())
        bk_sb = cpool.tile([128, 4], F32, name="bk", tag="bk")
        dma(out=bk_sb[:], in_=d["bk"].ap())
        bv_sb = cpool.tile([128, 520], F32, name="bv", tag="bv")
        dma(out=bv_sb[:], in_=d["bv"].ap())

        qT = [persist.tile([128, T], BF16, name=f"qT{p}", tag=f"qT{p}") for p in range(4)]
        kTp = [
            [
                persist.tile([128, T], BF16, name=f"kT{p}_{h2}", tag=f"kT{p}_{h2}")
                for h2 in (0, 1)
            ]
            for p in range(4)
        ]
        Vt = [persist.tile([128, 520], BF16, name=f"V{i}", tag=f"V{i}") for i in range(TT)]
        for p in range(4):
            nc.gpsimd.memset(kTp[p][0][64:128, :], 0.0)
            nc.gpsimd.memset(kTp[p][1][0:64, :], 0.0)

        # ---- phase 1+2: q^T/k^T head-pair tiles + V (shared x loads) ----
        with (
            tc.tile_pool(name="wqkv", bufs=1) as wpool,
            tc.tile_pool(name="xt1", bufs=2) as xpool,
            tc.tile_pool(name="psqk", bufs=3, space="PSUM") as pq,
            tc.tile_pool(name="psv", bufs=3, space="PSUM") as pv,
        ):
            xts = []
            for tci in range(2):
                xt = xpool.tile([128, CT, 512], F32R, name="xt", tag="xt")
                for cg in range(2):
                    dma(
                        out=xt[:, 4 * cg : 4 * cg + 4, :],
                        in_=d["xT"]
                        .ap()[512 * cg : 512 * cg + 512, 512 * tci : 512 * tci + 512]
                        .rearrange("(ct p) n -> p ct n", p=128),
                    )
                xts.append(xt)
            wq_sb = wpool.tile([128, CT, 512], F32R, name="wq", tag="wq")
            wk_sb = wpool.tile([128, CT, 512], F32R, name="wk", tag="wk")
            wv_sb = wpool.tile([128, CT, 520], F32R, name="wv", tag="wv")
            for cg in range(2):
                dma(
                    out=wq_sb[:, 4 * cg : 4 * cg + 4, :],
                    in_=d["wq"].ap()[512 * cg : 512 * cg + 512, :].rearrange(
                        "(ct p) n -> p ct n", p=128
                    ),
                )
                dma(
                    out=wk_sb[:, 4 * cg : 4 * cg + 4, :],
                    in_=d["wk"].ap()[512 * cg : 512 * cg + 512, :].rearrange(
                        "(ct p) n -> p ct n", p=128
                    ),
                )
                dma(
                    out=wv_sb[:, 4 * cg : 4 * cg + 4, :],
                    in_=d["wv"].ap()[512 * cg : 512 * cg + 512, :].rearrange(
                        "(ct p) n -> p ct n", p=128
                    ),
                )
            for tci in range(TC):
                if tci < 2:
                    xt = xts[tci]
                else:
                    xt = xpool.tile([128, CT, 512], F32R, name="xt", tag="xt")
                    dma(
                        out=xt[:],
                        in_=d["xT"].ap()[:, 512 * tci : 512 * tci + 512].rearrange(
                            "(ct p) n -> p ct n", p=128
                        ),
                    )
                for p in range(4):
                    for iw, w_sb in ((0, wq_sb), (1, wk_sb)):
                        ps = pq.tile([128, 512], F32, name="psqk", tag="psqk")
                        for ct in range(CT):
                            nc.tensor.matmul(
                                ps[:],
                                w_sb[:, ct, 128 * p : 128 * p + 128],
                                xt[:, ct, :],
                                start=(ct == 0),
                                stop=(ct == CT - 1),
                            )
                        if iw == 0:
                            nc.vector.tensor_scalar(
                                qT[p][:, 512 * tci : 512 * tci + 512],
                                ps[:],
                                0.125,
                                bq_sb[:, p : p + 1],
                                mult,
                                add,
                            )
                        else:
                            for h2 in (0, 1):
                                pr = 64 * h2
                                nc.vector.tensor_scalar(
                                    kTp[p][h2][
                                        pr : pr + 64, 512 * tci : 512 * tci + 512
                                    ],
                                    ps[pr : pr + 64, :],
                                    1.0,
                                    bk_sb[pr : pr + 64, p : p + 1],
                                    mult,
                                    add,
                                )
                for tt in range(4):
                    for qd in range(2):
                        ps = pv.tile([128, 260], F32, name="psv", tag="psv")
                        for ct in range(CT):
                            nc.tensor.matmul(
                                ps[:],
                                xt[:, ct, 128 * tt : 128 * tt + 128],
                                wv_sb[:, ct, 260 * qd : 260 * qd + 260],
                                start=(ct == 0),
                                stop=(ct == CT - 1),
                            )
                        nc.vector.tensor_tensor(
                            Vt[4 * tci + tt][:, 260 * qd : 260 * qd + 260],
                            ps[:],
                            bv_sb[:, 260 * qd : 260 * qd + 260],
                            add,
                        )

        wppool = tc.alloc_tile_pool(name="wpj", bufs=1)
        wp_sb = wppool.tile([128, 4, 1024], F32R, name="wp", tag="wp")
        dma(out=wp_sb[:], in_=d["wproj"].ap().rearrange("(pp p) n -> p pp n", p=128))
        ypool = tc.alloc_tile_pool(name="ypool", bufs=1)
        yT = [ypool.tile([128, T], F32R, name=f"yT{p}", tag=f"yT{p}") for p in range(4)]

        # ---- phase 3: attention in S^T layout, head pairs, K=128 via padded kT ----
        with (
            tc.tile_pool(name="es", bufs=6) as espool,
            tc.tile_pool(name="rc", bufs=4) as rcpool,
            tc.tile_pool(name="pss", bufs=2, space="PSUM") as pss,
            tc.tile_pool(name="psy", bufs=2, space="PSUM") as psy,
        ):
            for p in range(4):
                pending = None  # deferred (yqs, qc) tail copies

                def flush_tail(pend, p=None):
                    yq_t, qc_t = pend
                    for h2 in (0, 1):
                        pr = 64 * h2
                        cs = 512 * qc_t
                        nc.vector.tensor_copy(
                            out=yT[p][pr : pr + 64, cs : cs + 512],
                            in_=yq_t[h2][0:64, :],
                        )
                        rsum = rcpool.tile([1, 512], F32, name="rsum", tag="rsum")
                        nc.vector.tensor_copy(out=rsum[:], in_=yq_t[h2][64:65, :])
                        rs8 = rcpool.tile([64, 8], F32, name="rs8", tag="rs8")
                        dma(out=rs8[:], in_=rsum[:])
                        rr8 = rcpool.tile([64, 8], F32, name="rr8", tag="rr8")
                        nc.vector.reciprocal(rr8[:], rs8[:])
                        dma(out=rsum[:], in_=rr8[:])
                        rcb = rcpool.tile([128, 512], F32, name="rcb", tag="rcb")
                        nc.gpsimd.partition_broadcast(rcb[:], rsum[:])
                        nc.vector.tensor_tensor(
                            yT[p][pr : pr + 64, cs : cs + 512],
                            yT[p][pr : pr + 64, cs : cs + 512],
                            rcb[pr : pr + 64, :],
                            mult,
                        )

                for qc in range(4):
                    yqs = [
                        psy.tile([65, 512], F32, name=f"yq{h2}", tag=f"yq{h2}")
                        for h2 in (0, 1)
                    ]
                    nki = 4 * qc + 4
                    for ki in range(nki):
                        sblk = pss.tile([128, 1024], F32, name="sblk", tag="sblk")
                        for h2 in (0, 1):
                            nc.tensor.matmul(
                                sblk[:, 512 * h2 : 512 * h2 + 512],
                                kTp[p][h2][:, 128 * ki : 128 * ki + 128],
                                qT[p][:, 512 * qc : 512 * qc + 512],
                                start=True,
                                stop=True,
                            )
                        r = ki - 4 * qc
                        es = espool.tile([128, 1024], BF16, name="es", tag="es")
                        if r >= 1:
                            # exp only the causally-reachable region of each half
                            w = 512 - 128 * r
                            sv = sblk[:, 128 * r : 128 * r + w]
                            s2 = bass.AP(
                                tensor=sv.tensor,
                                offset=sv.offset,
                                ap=[list(sv.ap[0]), [512, 2], list(sv.ap[1])],
                            )
                            ev = es[:, 128 * r : 128 * r + w]
                            e2 = bass.AP(
                                tensor=ev.tensor,
                                offset=ev.offset,
                                ap=[list(ev.ap[0]), [512, 2], list(ev.ap[1])],
                            )
                            nc.scalar.activation(e2, s2, Exp)
                        else:
                            nc.scalar.activation(es[:], sblk[:], Exp)
                        if r >= 0:
                            # zero the upper triangle of the 128-wide boundary block
                            for h2 in (0, 1):
                                c0 = 512 * h2 + 128 * r
                                nc.vector.tensor_tensor(
                                    es[:, c0 : c0 + 128],
                                    es[:, c0 : c0 + 128],
                                    m01_sb[:],
                                    mult,
                                )
                        first, last = ki == 0, ki == nki - 1
                        vs = max(0, 128 * r)
                        for h2 in (0, 1):
                            hl = 2 * p + h2
                            nc.tensor.matmul(
                                yqs[h2][:, vs:512],
                                Vt[ki][:, 65 * hl : 65 * hl + 65],
                                es[:, 512 * h2 + vs : 512 * h2 + 512],
                                start=first,
                                stop=last,
                                skip_group_check=True,
                            )
                        if ki == 0 and pending is not None:
                            flush_tail(pending, p)
                            pending = None
                    pending = (yqs, qc)
                flush_tail(pending, p)
                pending = None

        # ---- phase 4: out-projection (row-sharded partial) ----
        with (
            tc.tile_pool(name="osb", bufs=3) as opool,
            tc.tile_pool(name="pso", bufs=4, space="PSUM") as pso,
        ):
            for tt in range(TT):
                for cc in range(2):
                    ps = pso.tile([128, 512], F32, name="pso", tag="pso")
                    for pp in range(4):
                        nc.tensor.matmul(
                            ps[:],
                            yT[pp][:, 128 * tt : 128 * tt + 128],
                            wp_sb[:, pp, 512 * cc : 512 * cc + 512],
                            start=(pp == 0),
                            stop=(pp == 3),
                        )
                    ob = opool.tile([128, 512], F32, name="ob", tag="ob")
                    nc.scalar.copy(ob[:], ps[:])
                    dma(
                        out=d["out"].ap()[
                            128 * tt : 128 * tt + 128, 512 * cc : 512 * cc + 512
                        ],
                        in_=ob[:],
                    )
        ypool.release()
        wppool.release()


def _build():
    nc = bacc.Bacc("TRN2", target_bir_lowering=False, debug=False, num_devices=N_CORES)
    d = {
        "xT": nc.dram_tensor("xT", [C, T], F32R, kind="ExternalInput"),
        "wq": nc.dram_tensor("wq", [C, 512], F32R, kind="ExternalInput"),
        "wk": nc.dram_tensor("wk", [C, 512], F32R, kind="ExternalInput"),
        "wv": nc.dram_tensor("wv", [C, 520], F32R, kind="ExternalInput"),
        "bv": nc.dram_tensor("bv", [128, 520], F32, kind="ExternalInput"),
        "bq": nc.dram_tensor("bq", [128, 4], F32, kind="ExternalInput"),
        "bk": nc.dram_tensor("bk", [128, 4], F32, kind="ExternalInput"),
        "m01": nc.dram_tensor("m01", [128, 128], mybir.dt.bfloat16, kind="ExternalInput"),
        "wproj": nc.dram_tensor("wproj", [512, 1024], F32R, kind="ExternalInput"),
        "out": nc.dram_tensor("out", [T, C], F32, kind="ExternalOutput"),
    }
    with tile.TileContext(nc) as tcx:
        _emit(nc, tcx, d)
    nc.compile()
    return nc


def _prep_core_inputs(c, x, w_attn, b_attn):
    g = c % 2
    xT = np.ascontiguousarray(x[c // 2].T)
    wq = np.ascontiguousarray(w_attn[:, 512 * g : 512 * g + 512])
    wk = np.ascontiguousarray(w_attn[:, 1024 + 512 * g : 1024 + 512 * g + 512])
    wv = np.zeros((C, 520), np.float32)
    bv = np.zeros((128, 520), np.float32)
    for hl in range(8):
        hcol = 2048 + 512 * g + 64 * hl
        wv[:, 65 * hl : 65 * hl + 64] = w_attn[:, hcol : hcol + 64]
        bv[:, 65 * hl : 65 * hl + 64] = b_attn[hcol : hcol + 64][None, :]
        bv[:, 65 * hl + 64] = 1.0
    bq = np.zeros((128, 4), np.float32)
    bk = np.zeros((128, 4), np.float32)
    for p in range(4):
        bq[:, p] = b_attn[512 * g + 128 * p : 512 * g + 128 * p + 128] * 0.125
        bk[:, p] = b_attn[1024 + 512 * g + 128 * p : 1024 + 512 * g + 128 * p + 128]
    import ml_dtypes

    m01 = (
        np.arange(128)[:, None] <= np.arange(128)[None, :]
    ).astype(ml_dtypes.bfloat16)
    return dict(xT=xT, wq=wq, wk=wk, wv=wv, bv=bv, bq=bq, bk=bk, m01=m01)


def make_in_maps(x, w_attn, b_attn, w_proj):
    x = np.asarray(x, np.float32)
    w_attn = np.asarray(w_attn, np.float32)
    b_attn = np.asarray(b_attn, np.float32)
    w_proj = np.asarray(w_proj, np.float32)
    in_maps = []
    for c in range(N_CORES):
        m = _prep_core_inputs(c, x, w_attn, b_attn)
        g = c % 2
        m["wproj"] = np.ascontiguousarray(w_proj[512 * g : 512 * g + 512, :])
        in_maps.append(m)
    return in_maps


def get_nc():
    if "nc" not in _cache:
        _cache["nc"] = _build()
    return _cache["nc"]


def gather(results, b_proj):
    b_proj = np.asarray(b_proj, np.float32)
    full = np.empty((B, T, C), np.float32)
    for b in range(B):
        full[b] = results[2 * b]["out"] + results[2 * b + 1]["out"] + b_proj[None, :]
    return full


def kernel(x, w_attn, b_attn, w_proj, b_proj):
    nc = get_nc()
    in_maps = make_in_maps(x, w_attn, b_attn, w_proj)
    res = run_bass_kernel_spmd(nc, in_maps, list(range(N_CORES)))
    return gather(res.results, b_proj)


# revision 21
# speedup vs baseline: 3.3367x; 1.0023x over previous
"""Causal self-attention (B=4, T=2048, C=1024, 16 heads) on 8 trn2 NeuronCores.

Sharding: core c = (batch c//2, head-group c%2 of 8 heads). Data-parallel over
batch, tensor-parallel over heads; out-proj is row-sharded and the two partial
products per batch are summed on the host (no device collectives).

Device program per core (all fp32):
  phase 1: q^T/k^T = W^T @ x^T as head-pair tiles [128, T] (d on partitions)
  phase 2: V in natural [t, d] layout, augmented with a ones column per head
           (bias trick) so P@V also accumulates softmax row-sums for free
  phase 3: flash-style attention in S^T layout (S computed transposed — no PE
           transposes, no max subtraction: |S| < ~3 by construction), causal
           masking via a single static 128x128 triangular mask + memsets,
           normalization folded into the PSUM->SBUF copy
  phase 4: out = y^T-slices^T @ W_proj rows (partial over this core's heads)
"""

import os
import sys

import numpy as np

for _p in ("/opt/trn_rl_repo", "/root/.axon_site/_ro/trn_rl_repo"):
    if os.path.isdir(_p) and _p not in sys.path:
        sys.path.insert(0, _p)

import concourse.bass as bass  # noqa: E402
import concourse.tile as tile  # noqa: E402
from concourse import bacc, mybir  # noqa: E402
from concourse.bass_utils import run_bass_kernel_spmd  # noqa: E402

B, T, C = 4, 2048, 1024
H, D = 16, 64
N_CORES = 8
F32 = mybir.dt.float32
F32R = mybir.dt.float32r
BF16 = mybir.dt.bfloat16


def _r(ap):
    return ap.bitcast(F32R)
TC = T // 512  # 4 t-chunks of 512
TT = T // 128  # 16 t-tiles of 128
CT = C // 128  # 8 c-tiles of 128

_cache: dict = {}


def _emit(nc: "bacc.Bacc", tc: "tile.TileContext", d: dict) -> None:
    mult = mybir.AluOpType.mult
    add = mybir.AluOpType.add
    Exp = mybir.ActivationFunctionType.Exp
    dma = nc.sync.dma_start

    with (
        tc.tile_pool(name="const", bufs=1) as cpool,
        tc.tile_pool(name="persist", bufs=1) as persist,
    ):
        m01_sb = cpool.tile([128, 128], BF16, name="m01", tag="m01")
        dma(out=m01_sb[:], in_=d["m01"].ap())
        warm_sb = cpool.tile([128, 2], F32, name="warm", tag="warm")
        nc.vector.memset(warm_sb[:, 0:1], 0.0)
        nc.scalar.activation(
            warm_sb[:, 1:2], warm_sb[:, 0:1], mybir.ActivationFunctionType.Exp
        )
        bq_sb = cpool.tile([128, 4], F32, name="bq", tag="bq")
        dma(out=bq_sb[:], in_=d["bq"].ap())
        bk_sb = cpool.tile([128, 4], F32, name="bk", tag="bk")
        dma(out=bk_sb[:], in_=d["bk"].ap())
        bv_sb = cpool.tile([128, 520], F32, name="bv", tag="bv")
        dma(out=bv_sb[:], in_=d["bv"].ap())

        qT = [persist.tile([128, T], BF16, name=f"qT{p}", tag=f"qT{p}") for p in range(4)]
        kTp = [
            [
                persist.tile([128, T], BF16, name=f"kT{p}_{h2}", tag=f"kT{p}_{h2}")
                for h2 in (0, 1)
            ]
            for p in range(4)
        ]
        Vt = [persist.tile([128, 520], BF16, name=f"V{i}", tag=f"V{i}") for i in range(TT)]
        for p in range(4):
            nc.gpsimd.memset(kTp[p][0][64:128, :], 0.0)
            nc.gpsimd.memset(kTp[p][1][0:64, :], 0.0)

        # ---- phase 1+2: q^T/k^T head-pair tiles + V (shared x loads) ----
        with (
            tc.tile_pool(name="wqkv", bufs=1) as wpool,
            tc.tile_pool(name="xt1", bufs=2) as xpool,
            tc.tile_pool(name="psqk", bufs=3, space="PSUM") as pq,
            tc.tile_pool(name="psv", bufs=3, space="PSUM") as pv,
        ):
            xts = []
            xt0 = xpool.tile([128, CT, 512], F32R, name="xt", tag="xt")
            wq_sb = wpool.tile([128, CT, 512], F32R, name="wq", tag="wq")
            wk_sb = wpool.tile([128, CT, 512], F32R, name="wk", tag="wk")
            wv_sb = wpool.tile([128, CT, 520], F32R, name="wv", tag="wv")
            # interleave per-c-tile pieces of xt chunk 0 / wq / wk so the first
            # psum group's dependencies arrive in consumption order across queues
            for ct in range(CT):
                dma(
                    out=xt0[:, ct, :],
                    in_=d["xT"]
                    .ap()[128 * ct : 128 * ct + 128, 0:512]
                    .rearrange("(o p) n -> p (o n)", p=128),
                )
                dma(
                    out=wq_sb[:, ct, :],
                    in_=d["wq"].ap()[128 * ct : 128 * ct + 128, :].rearrange(
                        "(o p) n -> p (o n)", p=128
                    ),
                )
                dma(
                    out=wk_sb[:, ct, :],
                    in_=d["wk"].ap()[128 * ct : 128 * ct + 128, :].rearrange(
                        "(o p) n -> p (o n)", p=128
                    ),
                )
            xts.append(xt0)
            xt1 = xpool.tile([128, CT, 512], F32R, name="xt", tag="xt")
            for cg in range(2):
                dma(
                    out=xt1[:, 4 * cg : 4 * cg + 4, :],
                    in_=d["xT"]
                    .ap()[512 * cg : 512 * cg + 512, 512:1024]
                    .rearrange("(ct p) n -> p ct n", p=128),
                )
                dma(
                    out=wv_sb[:, 4 * cg : 4 * cg + 4, :],
                    in_=d["wv"].ap()[512 * cg : 512 * cg + 512, :].rearrange(
                        "(ct p) n -> p ct n", p=128
                    ),
                )
            xts.append(xt1)
            for tci in range(TC):
                if tci < 2:
                    xt = xts[tci]
                else:
                    xt = xpool.tile([128, CT, 512], F32R, name="xt", tag="xt")
                    dma(
                        out=xt[:],
                        in_=d["xT"].ap()[:, 512 * tci : 512 * tci + 512].rearrange(
                            "(ct p) n -> p ct n", p=128
                        ),
                    )
                for p in range(4):
                    for iw, w_sb in ((0, wq_sb), (1, wk_sb)):
                        ps = pq.tile([128, 512], F32, name="psqk", tag="psqk")
                        for ct in range(CT):
                            nc.tensor.matmul(
                                ps[:],
                                w_sb[:, ct, 128 * p : 128 * p + 128],
                                xt[:, ct, :],
                                start=(ct == 0),
                                stop=(ct == CT - 1),
                            )
                        if iw == 0:
                            nc.vector.tensor_scalar(
                                qT[p][:, 512 * tci : 512 * tci + 512],
                                ps[:],
                                0.125,
                                bq_sb[:, p : p + 1],
                                mult,
                                add,
                            )
                        else:
                            for h2 in (0, 1):
                                pr = 64 * h2
                                nc.vector.tensor_scalar(
                                    kTp[p][h2][
                                        pr : pr + 64, 512 * tci : 512 * tci + 512
                                    ],
                                    ps[pr : pr + 64, :],
                                    1.0,
                                    bk_sb[pr : pr + 64, p : p + 1],
                                    mult,
                                    add,
                                )
                for tt in range(4):
                    for qd in range(2):
                        ps = pv.tile([128, 260], F32, name="psv", tag="psv")
                        for ct in range(CT):
                            nc.tensor.matmul(
                                ps[:],
                                xt[:, ct, 128 * tt : 128 * tt + 128],
                                wv_sb[:, ct, 260 * qd : 260 * qd + 260],
                                start=(ct == 0),
                                stop=(ct == CT - 1),
                            )
                        nc.vector.tensor_tensor(
                            Vt[4 * tci + tt][:, 260 * qd : 260 * qd + 260],
                            ps[:],
                            bv_sb[:, 260 * qd : 260 * qd + 260],
                            add,
                        )

        wppool = tc.alloc_tile_pool(name="wpj", bufs=1)
        wp_sb = wppool.tile([128, 4, 1024], F32R, name="wp", tag="wp")
        dma(out=wp_sb[:], in_=d["wproj"].ap().rearrange("(pp p) n -> p pp n", p=128))
        ypool = tc.alloc_tile_pool(name="ypool", bufs=1)
        yT = [ypool.tile([128, T], F32R, name=f"yT{p}", tag=f"yT{p}") for p in range(4)]

        # ---- phase 3: attention in S^T layout, head pairs, K=128 via padded kT ----
        with (
            tc.tile_pool(name="es", bufs=8) as espool,
            tc.tile_pool(name="rc", bufs=4) as rcpool,
            tc.tile_pool(name="pss", bufs=2, space="PSUM") as pss,
            tc.tile_pool(name="psy", bufs=2, space="PSUM") as psy,
        ):
            for p in range(4):
                pending = None  # deferred (yqs, qc) tail copies

                def flush_tail(pend, p=None):
                    yq_t, qc_t = pend
                    for h2 in (0, 1):
                        pr = 64 * h2
                        cs = 512 * qc_t
                        nc.vector.tensor_copy(
                            out=yT[p][pr : pr + 64, cs : cs + 512],
                            in_=yq_t[h2][0:64, :],
                        )
                        rsum = rcpool.tile([1, 512], F32, name="rsum", tag="rsum")
                        nc.vector.tensor_copy(out=rsum[:], in_=yq_t[h2][64:65, :])
                        rs8 = rcpool.tile([64, 8], F32, name="rs8", tag="rs8")
                        dma(out=rs8[:], in_=rsum[:])
                        rr8 = rcpool.tile([64, 8], F32, name="rr8", tag="rr8")
                        nc.vector.reciprocal(rr8[:], rs8[:])
                        dma(out=rsum[:], in_=rr8[:])
                        rcb = rcpool.tile([128, 512], F32, name="rcb", tag="rcb")
                        nc.gpsimd.partition_broadcast(rcb[:], rsum[:])
                        nc.vector.tensor_tensor(
                            yT[p][pr : pr + 64, cs : cs + 512],
                            yT[p][pr : pr + 64, cs : cs + 512],
                            rcb[pr : pr + 64, :],
                            mult,
                        )

                for qc in range(4):
                    yqs = [
                        psy.tile([65, 512], F32, name=f"yq{h2}", tag=f"yq{h2}")
                        for h2 in (0, 1)
                    ]
                    nki = 4 * qc + 4
                    for ki in range(nki):
                        sblk = pss.tile([128, 1024], F32, name="sblk", tag="sblk")
                        for h2 in (0, 1):
                            nc.tensor.matmul(
                                sblk[:, 512 * h2 : 512 * h2 + 512],
                                kTp[p][h2][:, 128 * ki : 128 * ki + 128],
                                qT[p][:, 512 * qc : 512 * qc + 512],
                                start=True,
                                stop=True,
                            )
                        r = ki - 4 * qc
                        es = espool.tile([128, 1024], BF16, name="es", tag="es")
                        if r >= 1:
                            # exp only the causally-reachable region of each half
                            w = 512 - 128 * r
                            sv = sblk[:, 128 * r : 128 * r + w]
                            s2 = bass.AP(
                                tensor=sv.tensor,
                                offset=sv.offset,
                                ap=[list(sv.ap[0]), [512, 2], list(sv.ap[1])],
                            )
                            ev = es[:, 128 * r : 128 * r + w]
                            e2 = bass.AP(
                                tensor=ev.tensor,
                                offset=ev.offset,
                                ap=[list(ev.ap[0]), [512, 2], list(ev.ap[1])],
                            )
                            nc.scalar.activation(e2, s2, Exp)
                        else:
                            nc.scalar.activation(es[:], sblk[:], Exp)
                        if r >= 0:
                            # zero the upper triangle of the 128-wide boundary block
                            for h2 in (0, 1):
                                c0 = 512 * h2 + 128 * r
                                nc.vector.tensor_tensor(
                                    es[:, c0 : c0 + 128],
                                    es[:, c0 : c0 + 128],
                                    m01_sb[:],
                                    mult,
                                )
                        first, last = ki == 0, ki == nki - 1
                        vs = max(0, 128 * r)
                        for h2 in (0, 1):
                            hl = 2 * p + h2
                            nc.tensor.matmul(
                                yqs[h2][:, vs:512],
                                Vt[ki][:, 65 * hl : 65 * hl + 65],
                                es[:, 512 * h2 + vs : 512 * h2 + 512],
                                start=first,
                                stop=last,
                                skip_group_check=True,
                            )
                        if ki == 0 and pending is not None:
                            flush_tail(pending, p)
                            pending = None
                    pending = (yqs, qc)
                flush_tail(pending, p)
                pending = None

        # ---- phase 4: out-projection (row-sharded partial) ----
        with (
            tc.tile_pool(name="osb", bufs=3) as opool,
            tc.tile_pool(name="pso", bufs=4, space="PSUM") as pso,
        ):
            for tt in range(TT):
                for cc in range(2):
                    ps = pso.tile([128, 512], F32, name="pso", tag="pso")
                    for pp in range(4):
                        nc.tensor.matmul(
                            ps[:],
                            yT[pp][:, 128 * tt : 128 * tt + 128],
                            wp_sb[:, pp, 512 * cc : 512 * cc + 512],
                            start=(pp == 0),
                            stop=(pp == 3),
                        )
                    ob = opool.tile([128, 512], F32, name="ob", tag="ob")
                    nc.scalar.copy(ob[:], ps[:])
                    dma(
                        out=d["out"].ap()[
                            128 * tt : 128 * tt + 128, 512 * cc : 512 * cc + 512
                        ],
                        in_=ob[:],
                    )
        ypool.release()
        wppool.release()


def _build():
    nc = bacc.Bacc("TRN2", target_bir_lowering=False, debug=False, num_devices=N_CORES)
    d = {
        "xT": nc.dram_tensor("xT", [C, T], F32R, kind="ExternalInput"),
        "wq": nc.dram_tensor("wq", [C, 512], F32R, kind="ExternalInput"),
        "wk": nc.dram_tensor("wk", [C, 512], F32R, kind="ExternalInput"),
        "wv": nc.dram_tensor("wv", [C, 520], F32R, kind="ExternalInput"),
        "bv": nc.dram_tensor("bv", [128, 520], F32, kind="ExternalInput"),
        "bq": nc.dram_tensor("bq", [128, 4], F32, kind="ExternalInput"),
        "bk": nc.dram_tensor("bk", [128, 4], F32, kind="ExternalInput"),
        "m01": nc.dram_tensor("m01", [128, 128], mybir.dt.bfloat16, kind="ExternalInput"),
        "wproj": nc.dram_tensor("wproj", [512, 1024], F32R, kind="ExternalInput"),
        "out": nc.dram_tensor("out", [T, C], F32, kind="ExternalOutput"),
    }
    with tile.TileContext(nc) as tcx:
        _emit(nc, tcx, d)
    nc.compile()
    return nc


def _prep_core_inputs(c, x, w_attn, b_attn):
    g = c % 2
    xT = np.ascontiguousarray(x[c // 2].T)
    wq = np.ascontiguousarray(w_attn[:, 512 * g : 512 * g + 512])
    wk = np.ascontiguousarray(w_attn[:, 1024 + 512 * g : 1024 + 512 * g + 512])
    wv = np.zeros((C, 520), np.float32)
    bv = np.zeros((128, 520), np.float32)
    for hl in range(8):
        hcol = 2048 + 512 * g + 64 * hl
        wv[:, 65 * hl : 65 * hl + 64] = w_attn[:, hcol : hcol + 64]
        bv[:, 65 * hl : 65 * hl + 64] = b_attn[hcol : hcol + 64][None, :]
        bv[:, 65 * hl + 64] = 1.0
    bq = np.zeros((128, 4), np.float32)
    bk = np.zeros((128, 4), np.float32)
    for p in range(4):
        bq[:, p] = b_attn[512 * g + 128 * p : 512 * g + 128 * p + 128] * 0.125
        bk[:, p] = b_attn[1024 + 512 * g + 128 * p : 1024 + 512 * g + 128 * p + 128]
    import ml_dtypes

    m01 = (
        np.arange(128)[:, None] <= np.arange(128)[None, :]
    ).astype(ml_dtypes.bfloat16)
    return dict(xT=xT, wq=wq, wk=wk, wv=wv, bv=bv, bq=bq, bk=bk, m01=m01)


def make_in_maps(x, w_attn, b_attn, w_proj):
    x = np.asarray(x, np.float32)
    w_attn = np.asarray(w_attn, np.float32)
    b_attn = np.asarray(b_attn, np.float32)
    w_proj = np.asarray(w_proj, np.float32)
    in_maps = []
    for c in range(N_CORES):
        m = _prep_core_inputs(c, x, w_attn, b_attn)
        g = c % 2
        m["wproj"] = np.ascontiguousarray(w_proj[512 * g : 512 * g + 512, :])
        in_maps.append(m)
    return in_maps


def get_nc():
    if "nc" not in _cache:
        _cache["nc"] = _build()
    return _cache["nc"]


def gather(results, b_proj):
    b_proj = np.asarray(b_proj, np.float32)
    full = np.empty((B, T, C), np.float32)
    for b in range(B):
        full[b] = results[2 * b]["out"] + results[2 * b + 1]["out"] + b_proj[None, :]
    return full


def kernel(x, w_attn, b_attn, w_proj, b_proj):
    nc = get_nc()
    in_maps = make_in_maps(x, w_attn, b_attn, w_proj)
    res = run_bass_kernel_spmd(nc, in_maps, list(range(N_CORES)))
    return gather(res.results, b_proj)


# revision 22
# speedup vs baseline: 3.4436x; 1.0321x over previous
"""Causal self-attention (B=4, T=2048, C=1024, 16 heads) on 8 trn2 NeuronCores.

Sharding: core c = (batch c//2, head-group c%2 of 8 heads). Data-parallel over
batch, tensor-parallel over heads; out-proj is row-sharded and the two partial
products per batch are summed on the host (no device collectives).

Device program per core (all fp32):
  phase 1: q^T/k^T = W^T @ x^T as head-pair tiles [128, T] (d on partitions)
  phase 2: V in natural [t, d] layout, augmented with a ones column per head
           (bias trick) so P@V also accumulates softmax row-sums for free
  phase 3: flash-style attention in S^T layout (S computed transposed — no PE
           transposes, no max subtraction: |S| < ~3 by construction), causal
           masking via a single static 128x128 triangular mask + memsets,
           normalization folded into the PSUM->SBUF copy
  phase 4: out = y^T-slices^T @ W_proj rows (partial over this core's heads)
"""

import os
import sys

import numpy as np

for _p in ("/opt/trn_rl_repo", "/root/.axon_site/_ro/trn_rl_repo"):
    if os.path.isdir(_p) and _p not in sys.path:
        sys.path.insert(0, _p)

import concourse.bass as bass  # noqa: E402
import concourse.tile as tile  # noqa: E402
from concourse import bacc, mybir  # noqa: E402
from concourse.bass_utils import run_bass_kernel_spmd  # noqa: E402

B, T, C = 4, 2048, 1024
H, D = 16, 64
N_CORES = 8
F32 = mybir.dt.float32
F32R = mybir.dt.float32r
BF16 = mybir.dt.bfloat16


def _r(ap):
    return ap.bitcast(F32R)
TC = T // 512  # 4 t-chunks of 512
TT = T // 128  # 16 t-tiles of 128
CT = C // 128  # 8 c-tiles of 128

_cache: dict = {}


def _emit(nc: "bacc.Bacc", tc: "tile.TileContext", d: dict) -> None:
    mult = mybir.AluOpType.mult
    add = mybir.AluOpType.add
    Exp = mybir.ActivationFunctionType.Exp
    dma = nc.sync.dma_start

    with (
        tc.tile_pool(name="const", bufs=1) as cpool,
        tc.tile_pool(name="persist", bufs=1) as persist,
    ):
        m01_sb = cpool.tile([128, 128], BF16, name="m01", tag="m01")
        dma(out=m01_sb[:], in_=d["m01"].ap())
        warm_sb = cpool.tile([128, 2], F32, name="warm", tag="warm")
        nc.vector.memset(warm_sb[:, 0:1], 0.0)
        nc.scalar.activation(
            warm_sb[:, 1:2], warm_sb[:, 0:1], mybir.ActivationFunctionType.Exp
        )
        bq_sb = cpool.tile([128, 4], F32, name="bq", tag="bq")
        dma(out=bq_sb[:], in_=d["bq"].ap())
        bk_sb = cpool.tile([128, 4], F32, name="bk", tag="bk")
        dma(out=bk_sb[:], in_=d["bk"].ap())
        bv_sb = cpool.tile([128, 520], F32, name="bv", tag="bv")
        dma(out=bv_sb[:], in_=d["bv"].ap())

        qT = [persist.tile([128, T], BF16, name=f"qT{p}", tag=f"qT{p}") for p in range(4)]
        kTp = [
            [
                persist.tile([128, T], BF16, name=f"kT{p}_{h2}", tag=f"kT{p}_{h2}")
                for h2 in (0, 1)
            ]
            for p in range(4)
        ]
        Vt = [persist.tile([128, 520], BF16, name=f"V{i}", tag=f"V{i}") for i in range(TT)]
        for p in range(4):
            nc.gpsimd.memset(kTp[p][0][64:128, :], 0.0)
            nc.gpsimd.memset(kTp[p][1][0:64, :], 0.0)

        # ---- phase 1+2: q^T/k^T head-pair tiles + V (shared x loads) ----
        with (
            tc.tile_pool(name="wqkv", bufs=1) as wpool,
            tc.tile_pool(name="xt1", bufs=2) as xpool,
            tc.tile_pool(name="psqk", bufs=3, space="PSUM") as pq,
            tc.tile_pool(name="psv", bufs=3, space="PSUM") as pv,
        ):
            xts = []
            xt0 = xpool.tile([128, CT, 512], F32R, name="xt", tag="xt")
            wq_sb = wpool.tile([128, CT, 512], F32R, name="wq", tag="wq")
            wk_sb = wpool.tile([128, CT, 512], F32R, name="wk", tag="wk")
            wv_sb = wpool.tile([128, CT, 520], F32R, name="wv", tag="wv")
            # interleave per-c-tile pieces of xt chunk 0 / wq / wk so the first
            # psum group's dependencies arrive in consumption order across queues
            for ct in range(CT):
                dma(
                    out=xt0[:, ct, :],
                    in_=d["xT"]
                    .ap()[128 * ct : 128 * ct + 128, 0:512]
                    .rearrange("(o p) n -> p (o n)", p=128),
                )
                dma(
                    out=wq_sb[:, ct, :],
                    in_=d["wq"].ap()[128 * ct : 128 * ct + 128, :].rearrange(
                        "(o p) n -> p (o n)", p=128
                    ),
                )
                dma(
                    out=wk_sb[:, ct, :],
                    in_=d["wk"].ap()[128 * ct : 128 * ct + 128, :].rearrange(
                        "(o p) n -> p (o n)", p=128
                    ),
                )
            xts.append(xt0)
            xt1 = xpool.tile([128, CT, 512], F32R, name="xt", tag="xt")
            for cg in range(2):
                dma(
                    out=xt1[:, 4 * cg : 4 * cg + 4, :],
                    in_=d["xT"]
                    .ap()[512 * cg : 512 * cg + 512, 512:1024]
                    .rearrange("(ct p) n -> p ct n", p=128),
                )
                dma(
                    out=wv_sb[:, 4 * cg : 4 * cg + 4, :],
                    in_=d["wv"].ap()[512 * cg : 512 * cg + 512, :].rearrange(
                        "(ct p) n -> p ct n", p=128
                    ),
                )
            xts.append(xt1)
            for tci in range(TC):
                if tci < 2:
                    xt = xts[tci]
                else:
                    xt = xpool.tile([128, CT, 512], F32R, name="xt", tag="xt")
                    dma(
                        out=xt[:],
                        in_=d["xT"].ap()[:, 512 * tci : 512 * tci + 512].rearrange(
                            "(ct p) n -> p ct n", p=128
                        ),
                    )
                for p in range(4):
                    for iw, w_sb in ((0, wq_sb), (1, wk_sb)):
                        ps = pq.tile([128, 512], F32, name="psqk", tag="psqk")
                        for ct in range(CT):
                            nc.tensor.matmul(
                                ps[:],
                                w_sb[:, ct, 128 * p : 128 * p + 128],
                                xt[:, ct, :],
                                start=(ct == 0),
                                stop=(ct == CT - 1),
                            )
                        if iw == 0:
                            nc.vector.tensor_scalar(
                                qT[p][:, 512 * tci : 512 * tci + 512],
                                ps[:],
                                0.125,
                                bq_sb[:, p : p + 1],
                                mult,
                                add,
                            )
                        else:
                            for h2 in (0, 1):
                                pr = 64 * h2
                                nc.vector.tensor_scalar(
                                    kTp[p][h2][
                                        pr : pr + 64, 512 * tci : 512 * tci + 512
                                    ],
                                    ps[pr : pr + 64, :],
                                    1.0,
                                    bk_sb[pr : pr + 64, p : p + 1],
                                    mult,
                                    add,
                                )
                for tt in range(4):
                    for qd in range(2):
                        ps = pv.tile([128, 260], F32, name="psv", tag="psv")
                        for ct in range(CT):
                            nc.tensor.matmul(
                                ps[:],
                                xt[:, ct, 128 * tt : 128 * tt + 128],
                                wv_sb[:, ct, 260 * qd : 260 * qd + 260],
                                start=(ct == 0),
                                stop=(ct == CT - 1),
                            )
                        nc.vector.tensor_tensor(
                            Vt[4 * tci + tt][:, 260 * qd : 260 * qd + 260],
                            ps[:],
                            bv_sb[:, 260 * qd : 260 * qd + 260],
                            add,
                        )

        wppool = tc.alloc_tile_pool(name="wpj", bufs=1)
        wp_sb = wppool.tile([128, 4, 1024], F32R, name="wp", tag="wp")
        dma(out=wp_sb[:], in_=d["wproj"].ap().rearrange("(pp p) n -> p pp n", p=128))
        ypool = tc.alloc_tile_pool(name="ypool", bufs=1)
        yT = [ypool.tile([128, T], F32R, name=f"yT{p}", tag=f"yT{p}") for p in range(4)]

        # ---- phase 3: attention in S^T layout, head pairs, K=128 via padded kT ----
        with (
            tc.tile_pool(name="es", bufs=8) as espool,
            tc.tile_pool(name="rc", bufs=4) as rcpool,
            tc.tile_pool(name="pss", bufs=3, space="PSUM") as pss,
            tc.tile_pool(name="psy", bufs=1, space="PSUM") as psy,
        ):
            for p in range(4):
                pending = None  # deferred (yqs, qc) tail copies

                def flush_tail(pend, p=None):
                    yq_t, qc_t = pend
                    for h2 in (0, 1):
                        pr = 64 * h2
                        cs = 512 * qc_t
                        nc.vector.tensor_copy(
                            out=yT[p][pr : pr + 64, cs : cs + 512],
                            in_=yq_t[h2][0:64, :],
                        )
                        rsum = rcpool.tile([1, 512], F32, name="rsum", tag="rsum")
                        nc.vector.tensor_copy(out=rsum[:], in_=yq_t[h2][64:65, :])
                        rs8 = rcpool.tile([64, 8], F32, name="rs8", tag="rs8")
                        dma(out=rs8[:], in_=rsum[:])
                        rr8 = rcpool.tile([64, 8], F32, name="rr8", tag="rr8")
                        nc.vector.reciprocal(rr8[:], rs8[:])
                        dma(out=rsum[:], in_=rr8[:])
                        rcb = rcpool.tile([128, 512], F32, name="rcb", tag="rcb")
                        nc.gpsimd.partition_broadcast(rcb[:], rsum[:])
                        nc.vector.tensor_tensor(
                            yT[p][pr : pr + 64, cs : cs + 512],
                            yT[p][pr : pr + 64, cs : cs + 512],
                            rcb[pr : pr + 64, :],
                            mult,
                        )

                for qc in range(4):
                    yqs = [
                        psy.tile([65, 512], F32, name=f"yq{h2}", tag=f"yq{h2}")
                        for h2 in (0, 1)
                    ]
                    nki = 4 * qc + 4
                    for ki in range(nki):
                        sblk = pss.tile([128, 1024], F32, name="sblk", tag="sblk")
                        for h2 in (0, 1):
                            nc.tensor.matmul(
                                sblk[:, 512 * h2 : 512 * h2 + 512],
                                kTp[p][h2][:, 128 * ki : 128 * ki + 128],
                                qT[p][:, 512 * qc : 512 * qc + 512],
                                start=True,
                                stop=True,
                            )
                        r = ki - 4 * qc
                        es = espool.tile([128, 1024], BF16, name="es", tag="es")
                        if r >= 1:
                            # exp only the causally-reachable region of each half
                            w = 512 - 128 * r
                            sv = sblk[:, 128 * r : 128 * r + w]
                            s2 = bass.AP(
                                tensor=sv.tensor,
                                offset=sv.offset,
                                ap=[list(sv.ap[0]), [512, 2], list(sv.ap[1])],
                            )
                            ev = es[:, 128 * r : 128 * r + w]
                            e2 = bass.AP(
                                tensor=ev.tensor,
                                offset=ev.offset,
                                ap=[list(ev.ap[0]), [512, 2], list(ev.ap[1])],
                            )
                            nc.scalar.activation(e2, s2, Exp)
                        else:
                            nc.scalar.activation(es[:], sblk[:], Exp)
                        if r >= 0:
                            # zero the upper triangle of the 128-wide boundary block
                            for h2 in (0, 1):
                                c0 = 512 * h2 + 128 * r
                                nc.vector.tensor_tensor(
                                    es[:, c0 : c0 + 128],
                                    es[:, c0 : c0 + 128],
                                    m01_sb[:],
                                    mult,
                                )
                        first, last = ki == 0, ki == nki - 1
                        vs = max(0, 128 * r)
                        for h2 in (0, 1):
                            hl = 2 * p + h2
                            nc.tensor.matmul(
                                yqs[h2][:, vs:512],
                                Vt[ki][:, 65 * hl : 65 * hl + 65],
                                es[:, 512 * h2 + vs : 512 * h2 + 512],
                                start=first,
                                stop=last,
                                skip_group_check=True,
                            )
                        if ki == 0 and pending is not None:
                            flush_tail(pending, p)
                            pending = None
                    pending = (yqs, qc)
                flush_tail(pending, p)
                pending = None

        # ---- phase 4: out-projection (row-sharded partial) ----
        with (
            tc.tile_pool(name="osb", bufs=3) as opool,
            tc.tile_pool(name="pso", bufs=4, space="PSUM") as pso,
        ):
            for tt in range(TT):
                for cc in range(2):
                    ps = pso.tile([128, 512], F32, name="pso", tag="pso")
                    for pp in range(4):
                        nc.tensor.matmul(
                            ps[:],
                            yT[pp][:, 128 * tt : 128 * tt + 128],
                            wp_sb[:, pp, 512 * cc : 512 * cc + 512],
                            start=(pp == 0),
                            stop=(pp == 3),
                        )
                    ob = opool.tile([128, 512], F32, name="ob", tag="ob")
                    nc.scalar.copy(ob[:], ps[:])
                    dma(
                        out=d["out"].ap()[
                            128 * tt : 128 * tt + 128, 512 * cc : 512 * cc + 512
                        ],
                        in_=ob[:],
                    )
        ypool.release()
        wppool.release()


def _build():
    nc = bacc.Bacc("TRN2", target_bir_lowering=False, debug=False, num_devices=N_CORES)
    d = {
        "xT": nc.dram_tensor("xT", [C, T], F32R, kind="ExternalInput"),
        "wq": nc.dram_tensor("wq", [C, 512], F32R, kind="ExternalInput"),
        "wk": nc.dram_tensor("wk", [C, 512], F32R, kind="ExternalInput"),
        "wv": nc.dram_tensor("wv", [C, 520], F32R, kind="ExternalInput"),
        "bv": nc.dram_tensor("bv", [128, 520], F32, kind="ExternalInput"),
        "bq": nc.dram_tensor("bq", [128, 4], F32, kind="ExternalInput"),
        "bk": nc.dram_tensor("bk", [128, 4], F32, kind="ExternalInput"),
        "m01": nc.dram_tensor("m01", [128, 128], mybir.dt.bfloat16, kind="ExternalInput"),
        "wproj": nc.dram_tensor("wproj", [512, 1024], F32R, kind="ExternalInput"),
        "out": nc.dram_tensor("out", [T, C], F32, kind="ExternalOutput"),
    }
    with tile.TileContext(nc) as tcx:
        _emit(nc, tcx, d)
    nc.compile()
    return nc


def _prep_core_inputs(c, x, w_attn, b_attn):
    g = c % 2
    xT = np.ascontiguousarray(x[c // 2].T)
    wq = np.ascontiguousarray(w_attn[:, 512 * g : 512 * g + 512])
    wk = np.ascontiguousarray(w_attn[:, 1024 + 512 * g : 1024 + 512 * g + 512])
    wv = np.zeros((C, 520), np.float32)
    bv = np.zeros((128, 520), np.float32)
    for hl in range(8):
        hcol = 2048 + 512 * g + 64 * hl
        wv[:, 65 * hl : 65 * hl + 64] = w_attn[:, hcol : hcol + 64]
        bv[:, 65 * hl : 65 * hl + 64] = b_attn[hcol : hcol + 64][None, :]
        bv[:, 65 * hl + 64] = 1.0
    bq = np.zeros((128, 4), np.float32)
    bk = np.zeros((128, 4), np.float32)
    for p in range(4):
        bq[:, p] = b_attn[512 * g + 128 * p : 512 * g + 128 * p + 128] * 0.125
        bk[:, p] = b_attn[1024 + 512 * g + 128 * p : 1024 + 512 * g + 128 * p + 128]
    import ml_dtypes

    m01 = (
        np.arange(128)[:, None] <= np.arange(128)[None, :]
    ).astype(ml_dtypes.bfloat16)
    return dict(xT=xT, wq=wq, wk=wk, wv=wv, bv=bv, bq=bq, bk=bk, m01=m01)


def make_in_maps(x, w_attn, b_attn, w_proj):
    x = np.asarray(x, np.float32)
    w_attn = np.asarray(w_attn, np.float32)
    b_attn = np.asarray(b_attn, np.float32)
    w_proj = np.asarray(w_proj, np.float32)
    in_maps = []
    for c in range(N_CORES):
        m = _prep_core_inputs(c, x, w_attn, b_attn)
        g = c % 2
        m["wproj"] = np.ascontiguousarray(w_proj[512 * g : 512 * g + 512, :])
        in_maps.append(m)
    return in_maps


def get_nc():
    if "nc" not in _cache:
        _cache["nc"] = _build()
    return _cache["nc"]


def gather(results, b_proj):
    b_proj = np.asarray(b_proj, np.float32)
    full = np.empty((B, T, C), np.float32)
    for b in range(B):
        full[b] = results[2 * b]["out"] + results[2 * b + 1]["out"] + b_proj[None, :]
    return full


def kernel(x, w_attn, b_attn, w_proj, b_proj):
    nc = get_nc()
    in_maps = make_in_maps(x, w_attn, b_attn, w_proj)
    res = run_bass_kernel_spmd(nc, in_maps, list(range(N_CORES)))
    return gather(res.results, b_proj)


# revision 23
# speedup vs baseline: 3.4808x; 1.0108x over previous
"""Causal self-attention (B=4, T=2048, C=1024, 16 heads) on 8 trn2 NeuronCores.

Sharding: core c = (batch c//2, head-group c%2 of 8 heads). Data-parallel over
batch, tensor-parallel over heads; out-proj is row-sharded and the two partial
products per batch are summed on the host (no device collectives).

Device program per core (all fp32):
  phase 1: q^T/k^T = W^T @ x^T as head-pair tiles [128, T] (d on partitions)
  phase 2: V in natural [t, d] layout, augmented with a ones column per head
           (bias trick) so P@V also accumulates softmax row-sums for free
  phase 3: flash-style attention in S^T layout (S computed transposed — no PE
           transposes, no max subtraction: |S| < ~3 by construction), causal
           masking via a single static 128x128 triangular mask + memsets,
           normalization folded into the PSUM->SBUF copy
  phase 4: out = y^T-slices^T @ W_proj rows (partial over this core's heads)
"""

import os
import sys

import numpy as np

for _p in ("/opt/trn_rl_repo", "/root/.axon_site/_ro/trn_rl_repo"):
    if os.path.isdir(_p) and _p not in sys.path:
        sys.path.insert(0, _p)

import concourse.bass as bass  # noqa: E402
import concourse.tile as tile  # noqa: E402
from concourse import bacc, mybir  # noqa: E402
from concourse.bass_utils import run_bass_kernel_spmd  # noqa: E402

B, T, C = 4, 2048, 1024
H, D = 16, 64
N_CORES = 8
F32 = mybir.dt.float32
F32R = mybir.dt.float32r
BF16 = mybir.dt.bfloat16


def _r(ap):
    return ap.bitcast(F32R)
TC = T // 512  # 4 t-chunks of 512
TT = T // 128  # 16 t-tiles of 128
CT = C // 128  # 8 c-tiles of 128

_cache: dict = {}


def _emit(nc: "bacc.Bacc", tc: "tile.TileContext", d: dict) -> None:
    mult = mybir.AluOpType.mult
    add = mybir.AluOpType.add
    Exp = mybir.ActivationFunctionType.Exp
    dma = nc.sync.dma_start

    with (
        tc.tile_pool(name="const", bufs=1) as cpool,
        tc.tile_pool(name="persist", bufs=1) as persist,
    ):
        m01_sb = cpool.tile([128, 128], BF16, name="m01", tag="m01")
        dma(out=m01_sb[:], in_=d["m01"].ap())
        warm_sb = cpool.tile([128, 2], F32, name="warm", tag="warm")
        nc.vector.memset(warm_sb[:, 0:1], 0.0)
        nc.scalar.activation(
            warm_sb[:, 1:2], warm_sb[:, 0:1], mybir.ActivationFunctionType.Exp
        )
        bq_sb = cpool.tile([128, 4], F32, name="bq", tag="bq")
        dma(out=bq_sb[:], in_=d["bq"].ap())
        bk_sb = cpool.tile([128, 4], F32, name="bk", tag="bk")
        dma(out=bk_sb[:], in_=d["bk"].ap())
        bv_sb = cpool.tile([128, 520], F32, name="bv", tag="bv")
        dma(out=bv_sb[:], in_=d["bv"].ap())

        qT = [persist.tile([128, T], BF16, name=f"qT{p}", tag=f"qT{p}") for p in range(4)]
        kTp = [
            [
                persist.tile([128, T], BF16, name=f"kT{p}_{h2}", tag=f"kT{p}_{h2}")
                for h2 in (0, 1)
            ]
            for p in range(4)
        ]
        Vt = [persist.tile([128, 520], BF16, name=f"V{i}", tag=f"V{i}") for i in range(TT)]
        for p in range(4):
            nc.gpsimd.memset(kTp[p][0][64:128, :], 0.0)
            nc.gpsimd.memset(kTp[p][1][0:64, :], 0.0)

        # ---- phase 1+2: q^T/k^T head-pair tiles + V (shared x loads) ----
        with (
            tc.tile_pool(name="wqkv", bufs=1) as wpool,
            tc.tile_pool(name="xt1", bufs=2) as xpool,
            tc.tile_pool(name="psqk", bufs=3, space="PSUM") as pq,
            tc.tile_pool(name="psv", bufs=3, space="PSUM") as pv,
        ):
            xts = []
            xt0 = xpool.tile([128, CT, 512], F32R, name="xt", tag="xt")
            wq_sb = wpool.tile([128, CT, 512], F32R, name="wq", tag="wq")
            wk_sb = wpool.tile([128, CT, 512], F32R, name="wk", tag="wk")
            wv_sb = wpool.tile([128, CT, 520], F32R, name="wv", tag="wv")
            # interleave per-c-tile pieces of xt chunk 0 / wq / wk so the first
            # psum group's dependencies arrive in consumption order across queues
            for ct in range(CT):
                dma(
                    out=xt0[:, ct, :],
                    in_=d["xT"]
                    .ap()[128 * ct : 128 * ct + 128, 0:512]
                    .rearrange("(o p) n -> p (o n)", p=128),
                )
                dma(
                    out=wq_sb[:, ct, :],
                    in_=d["wq"].ap()[128 * ct : 128 * ct + 128, :].rearrange(
                        "(o p) n -> p (o n)", p=128
                    ),
                )
                dma(
                    out=wk_sb[:, ct, :],
                    in_=d["wk"].ap()[128 * ct : 128 * ct + 128, :].rearrange(
                        "(o p) n -> p (o n)", p=128
                    ),
                )
            xts.append(xt0)
            xt1 = xpool.tile([128, CT, 512], F32R, name="xt", tag="xt")
            for cg in range(2):
                dma(
                    out=xt1[:, 4 * cg : 4 * cg + 4, :],
                    in_=d["xT"]
                    .ap()[512 * cg : 512 * cg + 512, 512:1024]
                    .rearrange("(ct p) n -> p ct n", p=128),
                )
                dma(
                    out=wv_sb[:, 4 * cg : 4 * cg + 4, :],
                    in_=d["wv"].ap()[512 * cg : 512 * cg + 512, :].rearrange(
                        "(ct p) n -> p ct n", p=128
                    ),
                )
            xts.append(xt1)
            for tci in range(TC):
                if tci < 2:
                    xt = xts[tci]
                else:
                    xt = xpool.tile([128, CT, 512], F32R, name="xt", tag="xt")
                    dma(
                        out=xt[:],
                        in_=d["xT"].ap()[:, 512 * tci : 512 * tci + 512].rearrange(
                            "(ct p) n -> p ct n", p=128
                        ),
                    )
                for p in range(4):
                    for iw, w_sb in ((0, wq_sb), (1, wk_sb)):
                        ps = pq.tile([128, 512], F32, name="psqk", tag="psqk")
                        for ct in range(CT):
                            nc.tensor.matmul(
                                ps[:],
                                w_sb[:, ct, 128 * p : 128 * p + 128],
                                xt[:, ct, :],
                                start=(ct == 0),
                                stop=(ct == CT - 1),
                            )
                        if iw == 0:
                            nc.vector.tensor_scalar(
                                qT[p][:, 512 * tci : 512 * tci + 512],
                                ps[:],
                                0.125,
                                bq_sb[:, p : p + 1],
                                mult,
                                add,
                            )
                        else:
                            for h2 in (0, 1):
                                pr = 64 * h2
                                nc.vector.tensor_scalar(
                                    kTp[p][h2][
                                        pr : pr + 64, 512 * tci : 512 * tci + 512
                                    ],
                                    ps[pr : pr + 64, :],
                                    1.0,
                                    bk_sb[pr : pr + 64, p : p + 1],
                                    mult,
                                    add,
                                )
                for tt in range(4):
                    for qd in range(2):
                        ps = pv.tile([128, 260], F32, name="psv", tag="psv")
                        for ct in range(CT):
                            nc.tensor.matmul(
                                ps[:],
                                xt[:, ct, 128 * tt : 128 * tt + 128],
                                wv_sb[:, ct, 260 * qd : 260 * qd + 260],
                                start=(ct == 0),
                                stop=(ct == CT - 1),
                            )
                        nc.vector.tensor_tensor(
                            Vt[4 * tci + tt][:, 260 * qd : 260 * qd + 260],
                            ps[:],
                            bv_sb[:, 260 * qd : 260 * qd + 260],
                            add,
                        )

        wppool = tc.alloc_tile_pool(name="wpj", bufs=1)
        wp_sb = wppool.tile([128, 4, 1024], F32R, name="wp", tag="wp")
        dma(out=wp_sb[:], in_=d["wproj"].ap().rearrange("(pp p) n -> p pp n", p=128))
        ypool = tc.alloc_tile_pool(name="ypool", bufs=1)
        yT = [ypool.tile([128, T], F32R, name=f"yT{p}", tag=f"yT{p}") for p in range(4)]

        # ---- phase 3: attention in S^T layout, head pairs, K=128 via padded kT ----
        with (
            tc.tile_pool(name="es", bufs=8) as espool,
            tc.tile_pool(name="rc", bufs=6) as rcpool,
            tc.tile_pool(name="pss", bufs=3, space="PSUM") as pss,
            tc.tile_pool(name="psy", bufs=1, space="PSUM") as psy,
        ):
            for p in range(4):
                pending = None  # deferred (yqs, qc) tail copies

                def flush_tail(pend, p=None):
                    yq_t, qc_t = pend
                    for h2 in (0, 1):
                        pr = 64 * h2
                        cs = 512 * qc_t
                        nc.vector.tensor_copy(
                            out=yT[p][pr : pr + 64, cs : cs + 512],
                            in_=yq_t[h2][0:64, :],
                        )
                        rsum = rcpool.tile([1, 512], F32, name="rsum", tag="rsum")
                        nc.vector.tensor_copy(out=rsum[:], in_=yq_t[h2][64:65, :])
                        rs8 = rcpool.tile([64, 8], F32, name="rs8", tag="rs8")
                        dma(out=rs8[:], in_=rsum[:])
                        rr8 = rcpool.tile([64, 8], F32, name="rr8", tag="rr8")
                        nc.vector.reciprocal(rr8[:], rs8[:])
                        dma(out=rsum[:], in_=rr8[:])
                        rcb = rcpool.tile([128, 512], F32, name="rcb", tag="rcb")
                        nc.gpsimd.partition_broadcast(rcb[:], rsum[:])
                        nc.vector.tensor_tensor(
                            yT[p][pr : pr + 64, cs : cs + 512],
                            yT[p][pr : pr + 64, cs : cs + 512],
                            rcb[pr : pr + 64, :],
                            mult,
                        )

                for qc in range(4):
                    yqs = [
                        psy.tile([65, 512], F32, name=f"yq{h2}", tag=f"yq{h2}")
                        for h2 in (0, 1)
                    ]
                    if pending is not None:
                        flush_tail(pending, p)
                        pending = None
                    nki = 4 * qc + 4
                    for ki in range(nki):
                        sblk = pss.tile([128, 1024], F32, name="sblk", tag="sblk")
                        for h2 in (0, 1):
                            nc.tensor.matmul(
                                sblk[:, 512 * h2 : 512 * h2 + 512],
                                kTp[p][h2][:, 128 * ki : 128 * ki + 128],
                                qT[p][:, 512 * qc : 512 * qc + 512],
                                start=True,
                                stop=True,
                            )
                        r = ki - 4 * qc
                        es = espool.tile([128, 1024], BF16, name="es", tag="es")
                        if r >= 1:
                            # exp only the causally-reachable region of each half
                            w = 512 - 128 * r
                            sv = sblk[:, 128 * r : 128 * r + w]
                            s2 = bass.AP(
                                tensor=sv.tensor,
                                offset=sv.offset,
                                ap=[list(sv.ap[0]), [512, 2], list(sv.ap[1])],
                            )
                            ev = es[:, 128 * r : 128 * r + w]
                            e2 = bass.AP(
                                tensor=ev.tensor,
                                offset=ev.offset,
                                ap=[list(ev.ap[0]), [512, 2], list(ev.ap[1])],
                            )
                            nc.scalar.activation(e2, s2, Exp)
                        else:
                            nc.scalar.activation(es[:], sblk[:], Exp)
                        if r >= 0:
                            # zero the upper triangle of the 128-wide boundary block
                            for h2 in (0, 1):
                                c0 = 512 * h2 + 128 * r
                                nc.vector.tensor_tensor(
                                    es[:, c0 : c0 + 128],
                                    es[:, c0 : c0 + 128],
                                    m01_sb[:],
                                    mult,
                                )
                        first, last = ki == 0, ki == nki - 1
                        vs = max(0, 128 * r)
                        for h2 in (0, 1):
                            hl = 2 * p + h2
                            nc.tensor.matmul(
                                yqs[h2][:, vs:512],
                                Vt[ki][:, 65 * hl : 65 * hl + 65],
                                es[:, 512 * h2 + vs : 512 * h2 + 512],
                                start=first,
                                stop=last,
                                skip_group_check=True,
                            )
                    pending = (yqs, qc)
                flush_tail(pending, p)
                pending = None

        # ---- phase 4: out-projection (row-sharded partial) ----
        with (
            tc.tile_pool(name="osb", bufs=3) as opool,
            tc.tile_pool(name="pso", bufs=4, space="PSUM") as pso,
        ):
            for tt in range(TT):
                for cc in range(2):
                    ps = pso.tile([128, 512], F32, name="pso", tag="pso")
                    for pp in range(4):
                        nc.tensor.matmul(
                            ps[:],
                            yT[pp][:, 128 * tt : 128 * tt + 128],
                            wp_sb[:, pp, 512 * cc : 512 * cc + 512],
                            start=(pp == 0),
                            stop=(pp == 3),
                        )
                    ob = opool.tile([128, 512], F32, name="ob", tag="ob")
                    nc.scalar.copy(ob[:], ps[:])
                    dma(
                        out=d["out"].ap()[
                            128 * tt : 128 * tt + 128, 512 * cc : 512 * cc + 512
                        ],
                        in_=ob[:],
                    )
        ypool.release()
        wppool.release()


def _build():
    nc = bacc.Bacc("TRN2", target_bir_lowering=False, debug=False, num_devices=N_CORES)
    d = {
        "xT": nc.dram_tensor("xT", [C, T], F32R, kind="ExternalInput"),
        "wq": nc.dram_tensor("wq", [C, 512], F32R, kind="ExternalInput"),
        "wk": nc.dram_tensor("wk", [C, 512], F32R, kind="ExternalInput"),
        "wv": nc.dram_tensor("wv", [C, 520], F32R, kind="ExternalInput"),
        "bv": nc.dram_tensor("bv", [128, 520], F32, kind="ExternalInput"),
        "bq": nc.dram_tensor("bq", [128, 4], F32, kind="ExternalInput"),
        "bk": nc.dram_tensor("bk", [128, 4], F32, kind="ExternalInput"),
        "m01": nc.dram_tensor("m01", [128, 128], mybir.dt.bfloat16, kind="ExternalInput"),
        "wproj": nc.dram_tensor("wproj", [512, 1024], F32R, kind="ExternalInput"),
        "out": nc.dram_tensor("out", [T, C], F32, kind="ExternalOutput"),
    }
    with tile.TileContext(nc) as tcx:
        _emit(nc, tcx, d)
    nc.compile()
    return nc


def _prep_core_inputs(c, x, w_attn, b_attn):
    g = c % 2
    xT = np.ascontiguousarray(x[c // 2].T)
    wq = np.ascontiguousarray(w_attn[:, 512 * g : 512 * g + 512])
    wk = np.ascontiguousarray(w_attn[:, 1024 + 512 * g : 1024 + 512 * g + 512])
    wv = np.zeros((C, 520), np.float32)
    bv = np.zeros((128, 520), np.float32)
    for hl in range(8):
        hcol = 2048 + 512 * g + 64 * hl
        wv[:, 65 * hl : 65 * hl + 64] = w_attn[:, hcol : hcol + 64]
        bv[:, 65 * hl : 65 * hl + 64] = b_attn[hcol : hcol + 64][None, :]
        bv[:, 65 * hl + 64] = 1.0
    bq = np.zeros((128, 4), np.float32)
    bk = np.zeros((128, 4), np.float32)
    for p in range(4):
        bq[:, p] = b_attn[512 * g + 128 * p : 512 * g + 128 * p + 128] * 0.125
        bk[:, p] = b_attn[1024 + 512 * g + 128 * p : 1024 + 512 * g + 128 * p + 128]
    import ml_dtypes

    m01 = (
        np.arange(128)[:, None] <= np.arange(128)[None, :]
    ).astype(ml_dtypes.bfloat16)
    return dict(xT=xT, wq=wq, wk=wk, wv=wv, bv=bv, bq=bq, bk=bk, m01=m01)


def make_in_maps(x, w_attn, b_attn, w_proj):
    x = np.asarray(x, np.float32)
    w_attn = np.asarray(w_attn, np.float32)
    b_attn = np.asarray(b_attn, np.float32)
    w_proj = np.asarray(w_proj, np.float32)
    in_maps = []
    for c in range(N_CORES):
        m = _prep_core_inputs(c, x, w_attn, b_attn)
        g = c % 2
        m["wproj"] = np.ascontiguousarray(w_proj[512 * g : 512 * g + 512, :])
        in_maps.append(m)
    return in_maps


def get_nc():
    if "nc" not in _cache:
        _cache["nc"] = _build()
    return _cache["nc"]


def gather(results, b_proj):
    b_proj = np.asarray(b_proj, np.float32)
    full = np.empty((B, T, C), np.float32)
    for b in range(B):
        full[b] = results[2 * b]["out"] + results[2 * b + 1]["out"] + b_proj[None, :]
    return full


def kernel(x, w_attn, b_attn, w_proj, b_proj):
    nc = get_nc()
    in_maps = make_in_maps(x, w_attn, b_attn, w_proj)
    res = run_bass_kernel_spmd(nc, in_maps, list(range(N_CORES)))
    return gather(res.results, b_proj)
